# revision 1
# baseline (speedup 1.0000x reference)
"""GAT (2-layer, 3-head) forward on 8 Trainium2 NeuronCores.

Math: with LeakyReLU slope ALPHA=1.0 the edge score e_ij = s1_i + s2_j is
linear, and s1_i cancels inside the row softmax.  The masked softmax over
j therefore reduces to column weights w_j = exp(s2_j - C) restricted to
adj, giving

    h'_i = (sum_j adj_ij * w_j * h_j) / (sum_j adj_ij * w_j)

i.e. one adjacency matmul against G = [w*h | w].  Both GAT layers take
this form (the same adjacency masks both), so the whole network is two
A-matmuls plus small projections.

Sharding: rows of h' (nodes) across 8 cores; each core holds lhsT-layout
adjacency columns A^T[:, slab] and computes its 512-row slab.  The
G matrices are built slab-wise and AllGathered per head so the gathers
pipeline against the adjacency matmul.  s2 = x @ (W a2) is computed up
front from host-folded u vectors so the max-reduction collective hides
under the x@W phase.  Matmuls run in bf16 hi/lo pair precision (~17
mantissa bits), accumulating in fp32 PSUM.
"""
import sys

sys.path.insert(0, "/opt/trn_rl_repo")

import numpy as np
import ml_dtypes

import concourse.bass as bass
import concourse.bacc as bacc
import concourse.mybir as mybir
import concourse.bass_isa as bass_isa
import concourse.tile as tile
from concourse.bass_utils import run_bass_kernel_spmd

BF16 = ml_dtypes.bfloat16

N = 4096
F = 768
HID = 768
NH = 3
NCLS = 256
NCORES = 8
SLAB = N // NCORES          # 512 rows per core
NIT = SLAB // 128           # 4 i-tiles per core
NJT = N // 128              # 32 j-tiles
NFT = F // 128              # 6 f-tiles
GH = NH * HID               # 2304 scaled-feature columns
NCT = NH * NFT              # 18 feature col-tiles of G
G2C = NCLS + 1              # 257 = classes + s2' column (folded u2)
PAD2 = 264                  # G2 half padded to 32B rows
WCOLS = 16                  # w-column slab width (3 used + pad)

AF = mybir.ActivationFunctionType
ALU = mybir.AluOpType


def _enable_ldw_opt():
    # walrus defaults to --enable-ldw-opt=false; with it off every LDWEIGHTS
    # serializes against the previous matmul (~427ns vs ~213ns per 512-col
    # matmul).  Patch the arg builder so the stationary loads pipeline.
    import concourse.bass_utils as _bu
    if getattr(_bu, "_ldw_opt_patched", False):
        return
    _orig = _bu.get_walrus_args

    def _patched(*a, **k):
        args = _orig(*a, **k)
        return [x.replace("--enable-ldw-opt=false", "--enable-ldw-opt=true")
                for x in args]

    _bu.get_walrus_args = _patched
    _bu._ldw_opt_patched = True


def build():
    dt = mybir.dt
    _enable_ldw_opt()
    nc = bacc.Bacc(num_devices=NCORES)

    adjT_d = nc.dram_tensor("adjT", [N, SLAB], dt.bfloat16, kind="ExternalInput")
    xTh_d = nc.dram_tensor("xT_hi", [F, SLAB], dt.bfloat16, kind="ExternalInput")
    xTl_d = nc.dram_tensor("xT_lo", [F, SLAB], dt.bfloat16, kind="ExternalInput")
    U6_d = nc.dram_tensor("U6", [F, 8], dt.bfloat16, kind="ExternalInput")
    U3_d = nc.dram_tensor("U3", [F, 8], dt.bfloat16, kind="ExternalInput")
    Wh_d = nc.dram_tensor("W_hi", [NH, F, HID], dt.bfloat16, kind="ExternalInput")
    Wl_d = nc.dram_tensor("W_lo", [NH, F, HID], dt.bfloat16, kind="ExternalInput")
    Woh_d = nc.dram_tensor("Wo_hi", [GH, G2C], dt.bfloat16, kind="ExternalInput")
    Wol_d = nc.dram_tensor("Wo_lo", [GH, G2C], dt.bfloat16, kind="ExternalInput")
    out_d = nc.dram_tensor("out", [SLAB, NCLS], dt.float32, kind="ExternalOutput")

    # DRAM scratch + collective buffers
    gs = [nc.dram_tensor(f"gs{h}", [SLAB, 2 * HID], dt.bfloat16) for h in range(NH)]
    gf = [nc.dram_tensor(f"gf{h}", [N, 2 * HID], dt.bfloat16, addr_space="Shared")
          for h in range(NH)]
    gsw = nc.dram_tensor("gsw", [SLAB, WCOLS], dt.bfloat16)
    gfw = nc.dram_tensor("gfw", [N, WCOLS], dt.bfloat16, addr_space="Shared")
    s2m_slab = nc.dram_tensor("s2m_slab", [8], dt.float32)
    s2m_full = nc.dram_tensor("s2m_full", [8 * NCORES], dt.float32, addr_space="Shared")
    s2p_slab = nc.dram_tensor("s2p_slab", [SLAB], dt.float32)
    s2p_full = nc.dram_tensor("s2p_full", [N], dt.float32, addr_space="Shared")
    g2_slab = nc.dram_tensor("g2_slab", [SLAB, 2 * PAD2], dt.bfloat16)
    g2_full = nc.dram_tensor("g2_full", [N, 2 * PAD2], dt.bfloat16, addr_space="Shared")

    rg = [list(range(NCORES))]

    with tile.TileContext(nc) as tc:
      with tc.tile_pool(name="adjt", bufs=NJT) as p_adjt:
        # ---------------- phase 1: s2, w, h=x@W, G build + gathers ----------
        with (
            tc.tile_pool(name="xw", bufs=1) as p_xw,
            tc.tile_pool(name="small", bufs=1) as p_sm,
            tc.tile_pool(name="gtmp", bufs=1) as p_gt,
        ):
            xhi, xlo = [], []
            xTh_t = xTh_d.rearrange("(ft p) i -> ft p i", p=128)
            xTl_t = xTl_d.rearrange("(ft p) i -> ft p i", p=128)
            for ft in range(NFT):
                t = p_xw.tile([128, SLAB], dt.bfloat16, tag="x", name="x", bufs=12)
                nc.sync.dma_start(t[:], xTh_t[ft])
                xhi.append(t)
                t = p_xw.tile([128, SLAB], dt.bfloat16, tag="x", name="x", bufs=12)
                nc.sync.dma_start(t[:], xTl_t[ft])
                xlo.append(t)
            u6 = p_sm.tile([128, NFT, 8], dt.bfloat16, tag="u6", name="u6")
            nc.sync.dma_start(u6[:], U6_d.rearrange("(ft p) c -> p ft c", p=128))
            u3 = p_sm.tile([128, NFT, 8], dt.bfloat16, tag="u3", name="u3")
            nc.sync.dma_start(u3[:], U3_d.rearrange("(ft p) c -> p ft c", p=128))

            # s2 = x @ u (tiny matmuls), slab max, tiny AllGather
            s2_sb = []
            for h in range(NH):
                s2_sb.append(p_sm.tile([128, NIT], dt.float32, tag="s2",
                                       name="s2", bufs=NH))
            with tc.tile_pool(name="psS", bufs=2, space="PSUM") as ps_s:
                for it in range(NIT):
                    p6 = ps_s.tile([128, 8], dt.float32, tag="p6", name="p6", bufs=2)
                    p3 = ps_s.tile([128, 8], dt.float32, tag="p3", name="p3", bufs=2)
                    for ft in range(NFT):
                        xh = xhi[ft][:, it * 128:(it + 1) * 128]
                        xl = xlo[ft][:, it * 128:(it + 1) * 128]
                        nc.tensor.matmul(p6[:], xh, u6[:, ft, :],
                                         start=(ft == 0), stop=(ft == NFT - 1))
                        nc.tensor.matmul(p3[:], xl, u3[:, ft, :],
                                         start=(ft == 0), stop=(ft == NFT - 1))
                    t6 = p_sm.tile([128, 8], dt.float32, tag="t6",
                                   name="t6", bufs=2)
                    nc.vector.tensor_copy(t6[:], p6[:])
                    tsum = p_sm.tile([128, NH], dt.float32, tag="tsum",
                                     name="tsum", bufs=2)
                    nc.vector.tensor_tensor(tsum[:], t6[:, 0:2 * NH:2],
                                            t6[:, 1:2 * NH:2], ALU.add)
                    for h in range(NH):
                        nc.vector.tensor_tensor(s2_sb[h][:, it:it + 1],
                                                tsum[:, h:h + 1], p3[:, h:h + 1],
                                                ALU.add)

            sm8 = p_sm.tile([1, 8], dt.float32, tag="sm8", name="sm8")
            nc.vector.memset(sm8[:], 0.0)
            for h in range(NH):
                m1 = p_sm.tile([128, 1], dt.float32, tag="m1", name="m1", bufs=2)
                nc.vector.tensor_reduce(m1[:], s2_sb[h][:],
                                        axis=mybir.AxisListType.X, op=ALU.max)
                m2 = p_sm.tile([128, 1], dt.float32, tag="m2", name="m2", bufs=2)
                nc.gpsimd.partition_all_reduce(m2[:], m1[:], channels=128,
                                               reduce_op=bass_isa.ReduceOp.max)
                nc.vector.tensor_copy(sm8[0:1, h:h + 1], m2[0:1, 0:1])
            nc.sync.dma_start(s2m_slab[:].rearrange("(o a) -> o a", o=1), sm8[:])
            nc.gpsimd.collective_compute(
                "AllGather", ALU.bypass, replica_groups=rg,
                ins=[s2m_slab[:]], outs=[s2m_full[:]])

            # W + adjacency loads overlap the collective latency
            whi = [[None] * NFT for _ in range(NH)]
            wlo = [[None] * NFT for _ in range(NH)]
            Wh_t = Wh_d.rearrange("h (ft p) o -> h ft p o", p=128)
            Wl_t = Wl_d.rearrange("h (ft p) o -> h ft p o", p=128)
            for h in range(NH):
                for ft in range(NFT):
                    t = p_xw.tile([128, HID], dt.bfloat16, tag="w", name="w", bufs=36)
                    nc.sync.dma_start(t[:], Wh_t[h, ft])
                    whi[h][ft] = t
                    t = p_xw.tile([128, HID], dt.bfloat16, tag="w", name="w", bufs=36)
                    nc.scalar.dma_start(t[:], Wl_t[h, ft])
                    wlo[h][ft] = t
            adjt = []
            adjT_t = adjT_d.rearrange("(jt p) i -> jt p i", p=128)
            for j in range(NJT):
                t = p_adjt.tile([128, SLAB], dt.bfloat16, tag="adjt", name="adjt")
                eng = nc.sync if j % 2 == 0 else nc.scalar
                eng.dma_start(t[:], adjT_t[j])
                adjt.append(t)

            mload = p_sm.tile([1, 8 * NCORES], dt.float32, tag="mload", name="mload")
            nc.sync.dma_start(mload[:], s2m_full[:].rearrange("(o a) -> o a", o=1))
            negC = p_sm.tile([1, NH], dt.float32, tag="negC", name="negC")
            for h in range(NH):
                nc.vector.tensor_reduce(
                    negC[0:1, h:h + 1], mload[0:1, h::8],
                    axis=mybir.AxisListType.X, op=ALU.max, negate=True)
            negCbc = p_sm.tile([128, NH], dt.float32, tag="negCbc", name="negCbc")
            nc.gpsimd.partition_broadcast(negCbc[:], negC[:], channels=128)

            w_sb = []
            for h in range(NH):
                w = p_sm.tile([128, NIT], dt.float32, tag="wexp", name="wexp", bufs=NH)
                nc.scalar.activation(w[:], s2_sb[h][:], AF.Exp,
                                     bias=negCbc[:, h:h + 1])
                w_sb.append(w)
            # bf16 pair of the w columns -> gsw slab -> tiny gather
            whi3 = p_sm.tile([128, NH, NIT], dt.bfloat16, tag="whi3", name="whi3")
            wlo3 = p_sm.tile([128, NH, NIT], dt.float32, tag="wlo3", name="wlo3")
            wlo3b = p_sm.tile([128, NH, NIT], dt.bfloat16, tag="wlo3b", name="wlo3b")
            for h in range(NH):
                nc.vector.tensor_copy(whi3[:, h, :], w_sb[h][:])
                nc.vector.tensor_tensor(wlo3[:, h, :], w_sb[h][:], whi3[:, h, :],
                                        ALU.subtract)
            nc.vector.tensor_copy(wlo3b[:], wlo3[:])
            for it in range(NIT):
                wt = p_sm.tile([128, WCOLS], dt.bfloat16, tag="wt", name="wt", bufs=2)
                nc.vector.memset(wt[:], 0.0)
                nc.vector.tensor_copy(wt[:, 0:NH], whi3[:, :, it])
                nc.vector.tensor_copy(wt[:, 8:8 + NH], wlo3b[:, :, it])
                nc.sync.dma_start(gsw[it * 128:(it + 1) * 128, :], wt[:])
            nc.gpsimd.collective_compute(
                "AllGather", ALU.bypass, replica_groups=rg,
                ins=[gsw[:]], outs=[gfw[:]])

            # h = x@W per head; scale by w; bf16 pair; per-head gather
            with tc.tile_pool(name="psA", bufs=4, space="PSUM") as ps_a:
                for h in range(NH):
                    for it in range(NIT):
                        ps = ps_a.tile([128, HID], dt.float32, tag="psA", name="psA")
                        c0 = c1 = 0
                        for ft in range(NFT):
                            xh = xhi[ft][:, it * 128:(it + 1) * 128]
                            xl = xlo[ft][:, it * 128:(it + 1) * 128]
                            for lhs, rhss in ((xh, (whi[h][ft], wlo[h][ft])),
                                              (xl, (whi[h][ft],))):
                                for rhs in rhss:
                                    nc.tensor.matmul(
                                        ps[:, 0:512], lhs, rhs[:, 0:512],
                                        start=(c0 == 0), stop=(c0 == 3 * NFT - 1))
                                    c0 += 1
                                    nc.tensor.matmul(
                                        ps[:, 512:HID], lhs, rhs[:, 512:HID],
                                        start=(c1 == 0), stop=(c1 == 3 * NFT - 1))
                                    c1 += 1
                        g = p_gt.tile([128, HID], dt.float32, tag="g", name="g",
                                      bufs=3)
                        nc.vector.tensor_scalar_mul(g[:], ps[:],
                                                    w_sb[h][:, it:it + 1])
                        ghi = p_gt.tile([128, HID], dt.bfloat16, tag="ghi",
                                        name="ghi", bufs=3)
                        glo32 = p_gt.tile([128, HID], dt.float32, tag="glo32",
                                          name="glo32", bufs=3)
                        glo = p_gt.tile([128, HID], dt.bfloat16, tag="glo",
                                        name="glo", bufs=3)
                        nc.vector.tensor_copy(ghi[:], g[:])
                        nc.vector.tensor_tensor(glo32[:], g[:], ghi[:], ALU.subtract)
                        nc.vector.tensor_copy(glo[:], glo32[:])
                        rows = slice(it * 128, (it + 1) * 128)
                        nc.sync.dma_start(gs[h][rows, 0:HID], ghi[:])
                        nc.sync.dma_start(gs[h][rows, HID:2 * HID], glo[:])
                    nc.gpsimd.collective_compute(
                        "AllGather", ALU.bypass, replica_groups=rg,
                        ins=[gs[h][:]], outs=[gf[h][:]])

        # ---------------- L1 adjacency matmul + epilogue + layer 2 ----------
        with tc.tile_pool(name="xct", bufs=1) as p_xct:
            with (
                tc.tile_pool(name="numt", bufs=2) as p_numt,
                tc.tile_pool(name="gst", bufs=12) as p_gst,
                tc.tile_pool(name="etmp", bufs=1) as p_et,
                tc.tile_pool(name="wo", bufs=1) as p_wo,
                tc.tile_pool(name="l2a", bufs=1) as p_l2a,
                tc.tile_pool(name="ps1", bufs=4, space="PSUM") as ps_1,
                tc.tile_pool(name="psh2", bufs=4, space="PSUM") as ps_h2,
            ):
                # denominator col-tile first: den_k = A @ w_k
                gwv = gfw.rearrange("(jt p) c -> p jt c", p=128)
                gwt = p_gst.tile([128, NJT, WCOLS], dt.bfloat16, tag="gwt",
                                 name="gwt", bufs=1)
                nc.sync.dma_start(gwt[:], gwv[:])
                psd = ps_1.tile([128, SLAB], dt.float32, tag="ps1", name="ps1")
                for j in range(NJT):
                    nc.tensor.matmul(psd[0:NH, :], gwt[:, j, 0:NH], adjt[j][:],
                                     start=(j == 0), stop=False)
                    nc.tensor.matmul(psd[0:NH, :], gwt[:, j, 8:8 + NH], adjt[j][:],
                                     start=False, stop=(j == NJT - 1))
                recip3 = p_et.tile([NH, SLAB], dt.float32, tag="recip3",
                                   name="recip3")
                nc.vector.reciprocal(recip3[:], psd[0:NH, :])
                rbc = []
                for h in range(NH):
                    rrow = p_et.tile([1, SLAB], dt.float32, tag="rrow",
                                     name="rrow", bufs=2)
                    nc.sync.dma_start(rrow[:], recip3[h:h + 1, :])
                    rb = p_et.tile([128, SLAB], dt.float32, tag="rbc",
                                   name="rbc", bufs=NH)
                    nc.gpsimd.partition_broadcast(rb[:], rrow[:], channels=128)
                    rbc.append(rb)

                # feature col-tiles, head-major; epilogue inline per ct
                xchi, xclo = [], []
                for ct in range(NCT):
                    h = ct // NFT
                    lct = ct % NFT
                    gv = gf[h].rearrange("(jb q p) (t c) -> jb p q t c",
                                         q=4, p=128, t=2)
                    ps = ps_1.tile([128, SLAB], dt.float32, tag="ps1", name="ps1")
                    for jb in range(NJT // 4):
                        gt = p_gst.tile([128, 4, 2, 128], dt.bfloat16, tag="gst",
                                        name="gst")
                        for tt in range(2):
                            eng = nc.sync if (jb + tt) % 2 == 0 else nc.scalar
                            eng.dma_start(gt[:, :, tt, :],
                                          gv[jb, :, :, tt,
                                             lct * 128:(lct + 1) * 128])
                        for q in range(4):
                            j = jb * 4 + q
                            nc.tensor.matmul(ps[:], gt[:, q, 0, :], adjt[j][:],
                                             start=(j == 0), stop=False)
                            nc.tensor.matmul(ps[:], gt[:, q, 1, :], adjt[j][:],
                                             start=False, stop=(j == NJT - 1))
                    # xcatT tile = elu(numT / den) and its bf16 pair
                    z = p_et.tile([128, SLAB], dt.float32, tag="z", name="z", bufs=2)
                    nc.vector.tensor_tensor(z[:], ps[:], rbc[h][:], ALU.mult)
                    e = p_et.tile([128, SLAB], dt.float32, tag="e", name="e", bufs=2)
                    nc.scalar.activation(e[:], z[:], AF.Exp)
                    nc.vector.tensor_scalar(e[:], e[:], 1.0, -1.0, ALU.min, ALU.add)
                    xc = p_et.tile([128, SLAB], dt.float32, tag="xc", name="xc",
                                   bufs=2)
                    nc.vector.scalar_tensor_tensor(xc[:], z[:], 0.0, e[:],
                                                   ALU.max, ALU.add)
                    th = p_xct.tile([128, SLAB], dt.bfloat16, tag="xcp",
                                    name="xcp", bufs=2 * NCT)
                    nc.vector.tensor_copy(th[:], xc[:])
                    r32 = p_et.tile([128, SLAB], dt.float32, tag="r32",
                                    name="r32", bufs=2)
                    nc.vector.tensor_tensor(r32[:], xc[:], th[:], ALU.subtract)
                    tl = p_xct.tile([128, SLAB], dt.bfloat16, tag="xcp",
                                    name="xcp", bufs=2 * NCT)
                    nc.vector.tensor_copy(tl[:], r32[:])
                    xchi.append(th)
                    xclo.append(tl)

                # Wo loads (needed only by the h2 matmul below; emitted
                # mid-ct-loop region so they don't clog the queues that feed
                # the L1 stationary prefetch)
                woh, wol = [], []
                Woh_t = Woh_d.rearrange("(ot p) c -> ot p c", p=128)
                Wol_t = Wol_d.rearrange("(ot p) c -> ot p c", p=128)
                for ot in range(NCT):
                    t = p_wo.tile([128, G2C], dt.bfloat16, tag="wo", name="wo",
                                  bufs=2 * NCT)
                    nc.sync.dma_start(t[:], Woh_t[ot])
                    woh.append(t)
                    t = p_wo.tile([128, G2C], dt.bfloat16, tag="wo", name="wo",
                                  bufs=2 * NCT)
                    nc.scalar.dma_start(t[:], Wol_t[ot])
                    wol.append(t)
                # layer 2: h2 = xcat@Wo (+ s2' via folded u2 column)
                ps2l = [ps_h2.tile([128, G2C], dt.float32, tag="psh2",
                                   name="psh2") for _ in range(NIT)]
                for ot in range(NCT):
                    for it in range(NIT):
                        xh = xchi[ot][:, it * 128:(it + 1) * 128]
                        xl = xclo[ot][:, it * 128:(it + 1) * 128]
                        nc.tensor.matmul(ps2l[it][:], xh, woh[ot][:],
                                         start=(ot == 0), stop=False)
                        nc.tensor.matmul(ps2l[it][:], xh, wol[ot][:],
                                         start=False, stop=False)
                        nc.tensor.matmul(ps2l[it][:], xl, woh[ot][:],
                                         start=False, stop=(ot == NCT - 1))
                h2_sb = []
                s2p = p_l2a.tile([128, NIT], dt.float32, tag="s2p", name="s2p")
                for it in range(NIT):
                    h2 = p_l2a.tile([128, NCLS], dt.float32, tag="h2", name="h2",
                                    bufs=NIT)
                    nc.vector.tensor_copy(h2[:], ps2l[it][:, 0:NCLS])
                    h2_sb.append(h2)
                    nc.vector.tensor_copy(s2p[:, it:it + 1],
                                          ps2l[it][:, NCLS:G2C])
                    nc.sync.dma_start(
                        s2p_slab[it * 128:(it + 1) * 128].rearrange(
                            "(p o) -> p o", o=1),
                        s2p[:, it:it + 1])
                nc.gpsimd.collective_compute(
                    "AllGather", ALU.bypass, replica_groups=rg,
                    ins=[s2p_slab[:]], outs=[s2p_full[:]])
                sload = p_l2a.tile([128, N // 128], dt.float32, tag="sload",
                                   name="sload")
                nc.sync.dma_start(sload[:],
                                  s2p_full[:].rearrange("(p a) -> p a", p=128))
                sm1 = p_l2a.tile([128, 1], dt.float32, tag="sm1", name="sm1")
                nc.vector.tensor_reduce(sm1[:], sload[:],
                                        axis=mybir.AxisListType.X, op=ALU.max)
                sm2 = p_l2a.tile([128, 1], dt.float32, tag="sm2", name="sm2")
                nc.gpsimd.partition_all_reduce(sm2[:], sm1[:], channels=128,
                                               reduce_op=bass_isa.ReduceOp.max)
                negC2 = p_l2a.tile([128, 1], dt.float32, tag="negC2", name="negC2")
                nc.vector.tensor_scalar_mul(negC2[:], sm2[:], -1.0)
                w2all = p_l2a.tile([128, NIT], dt.float32, tag="w2all", name="w2all")
                nc.scalar.activation(w2all[:], s2p[:], AF.Exp, bias=negC2[:])
                for it in range(NIT):
                    rows = slice(it * 128, (it + 1) * 128)
                    g2 = p_l2a.tile([128, PAD2], dt.float32, tag="g2", name="g2",
                                    bufs=2)
                    nc.vector.tensor_scalar_mul(g2[:, 0:NCLS], h2_sb[it][:],
                                                w2all[:, it:it + 1])
                    nc.vector.tensor_copy(g2[:, NCLS:G2C], w2all[:, it:it + 1])
                    nc.vector.memset(g2[:, G2C:PAD2], 0.0)
                    g2h = p_l2a.tile([128, PAD2], dt.bfloat16, tag="g2h",
                                     name="g2h", bufs=2)
                    g2r = p_l2a.tile([128, PAD2], dt.float32, tag="g2r",
                                     name="g2r", bufs=2)
                    g2l = p_l2a.tile([128, PAD2], dt.bfloat16, tag="g2l",
                                     name="g2l", bufs=2)
                    nc.vector.tensor_copy(g2h[:], g2[:])
                    nc.vector.tensor_tensor(g2r[:], g2[:], g2h[:], ALU.subtract)
                    nc.vector.tensor_copy(g2l[:], g2r[:])
                    nc.sync.dma_start(g2_slab[rows, 0:PAD2], g2h[:])
                    nc.sync.dma_start(g2_slab[rows, PAD2:2 * PAD2], g2l[:])
                nc.gpsimd.collective_compute(
                    "AllGather", ALU.bypass, replica_groups=rg,
                    ins=[g2_slab[:]], outs=[g2_full[:]])

            # L2 adjacency matmul + final epilogue
            with (
                tc.tile_pool(name="g2t", bufs=NJT) as p_g2t,
                tc.tile_pool(name="fin", bufs=1) as p_f,
                tc.tile_pool(name="ps2", bufs=4, space="PSUM") as ps_2,
            ):
                g2v = g2_full.rearrange("(jt p) (t c) -> jt p t c", p=128, t=2)
                g2tiles = []
                for j in range(NJT):
                    gt2 = p_g2t.tile([128, 2, PAD2], dt.bfloat16, tag="g2t",
                                     name="g2t")
                    eng = nc.sync if j % 2 == 0 else nc.scalar
                    eng.dma_start(gt2[:, :, 0:G2C], g2v[j, :, :, 0:G2C])
                    g2tiles.append(gt2)
                ps2 = [ps_2.tile([128, G2C], dt.float32, tag="ps2", name="ps2")
                       for _ in range(NIT)]
                for it in range(NIT):
                    for j in range(NJT):
                        lhs = adjt[j][:, it * 128:(it + 1) * 128]
                        nc.tensor.matmul(ps2[it][:], lhs,
                                         g2tiles[j][:, 0, 0:G2C],
                                         start=(j == 0), stop=False)
                        nc.tensor.matmul(ps2[it][:], lhs,
                                         g2tiles[j][:, 1, 0:G2C],
                                         start=False, stop=(j == NJT - 1))
                for it in range(NIT):
                    r2 = p_f.tile([128, 1], dt.float32, tag="r2", name="r2", bufs=2)
                    nc.vector.reciprocal(r2[:], ps2[it][:, NCLS:G2C])
                    z = p_f.tile([128, NCLS], dt.float32, tag="z2", name="z2",
                                 bufs=2)
                    nc.vector.tensor_scalar_mul(z[:], ps2[it][:, 0:NCLS], r2[:])
                    e = p_f.tile([128, NCLS], dt.float32, tag="e2", name="e2",
                                 bufs=2)
                    nc.scalar.activation(e[:], z[:], AF.Exp)
                    nc.vector.tensor_scalar(e[:], e[:], 1.0, -1.0, ALU.min, ALU.add)
                    o = p_f.tile([128, NCLS], dt.float32, tag="o2", name="o2",
                                 bufs=2)
                    nc.vector.scalar_tensor_tensor(o[:], z[:], 0.0, e[:],
                                                   ALU.max, ALU.add)
                    negm = p_f.tile([128, 1], dt.float32, tag="negm", name="negm",
                                    bufs=2)
                    nc.vector.tensor_reduce(negm[:], o[:],
                                            axis=mybir.AxisListType.X,
                                            op=ALU.max, negate=True)
                    t = p_f.tile([128, NCLS], dt.float32, tag="texp", name="texp",
                                 bufs=2)
                    nc.scalar.activation(t[:], o[:], AF.Exp, bias=negm[:])
                    ssum = p_f.tile([128, 1], dt.float32, tag="ssum", name="ssum",
                                    bufs=2)
                    nc.vector.tensor_reduce(ssum[:], t[:],
                                            axis=mybir.AxisListType.X, op=ALU.add)
                    lg = p_f.tile([128, 1], dt.float32, tag="lg", name="lg", bufs=2)
                    nc.scalar.activation(lg[:], ssum[:], AF.Ln)
                    fin = p_f.tile([128, NCLS], dt.float32, tag="fin", name="fin",
                                   bufs=2)
                    nc.vector.tensor_scalar(fin[:], o[:], negm[:], lg[:],
                                            ALU.add, ALU.subtract)
                    nc.sync.dma_start(out_d[it * 128:(it + 1) * 128, :], fin[:])

    nc.finalize()
    return nc


_CACHE = {}


def _pair(a):
    hi = a.astype(BF16)
    lo = (a - hi.astype(np.float32)).astype(BF16)
    return hi, lo


def prepare_inputs(x, adj, W_heads, a_heads, W_out, a_out):
    """Shard + lay out the full inputs for the 8 cores."""
    x2 = np.asarray(x, np.float32)[0]          # [N, F]
    adj2 = np.asarray(adj)[0]                  # [N, N] int32
    W3 = np.asarray(W_heads, np.float32).reshape(NH, F, HID)
    a3 = np.asarray(a_heads, np.float32)       # [NH, 2*HID, 1]
    Wo = np.asarray(W_out, np.float32).reshape(GH, NCLS)
    ao = np.asarray(a_out, np.float32)         # [2*NCLS, 1]

    # fold the edge-score projections into the weights:
    #   s2 = x @ (W @ a2),   s2' = xcat @ (Wo @ ao2)
    u = np.einsum("hfo,ho->hf", W3.astype(np.float64),
                  a3[:, HID:, 0].astype(np.float64)).astype(np.float32)  # [NH,F]
    u_hi, u_lo = _pair(u)
    U6 = np.zeros((F, 8), BF16)
    U3 = np.zeros((F, 8), BF16)
    for h in range(NH):
        U6[:, 2 * h] = u_hi[h]
        U6[:, 2 * h + 1] = u_lo[h]
        U3[:, h] = u_hi[h]
    u2 = (Wo.astype(np.float64) @ ao[NCLS:, 0].astype(np.float64)).astype(np.float32)
    Wo_ext = np.concatenate([Wo, u2[:, None]], axis=1)       # [GH, 257]
    Woh, Wol = _pair(Wo_ext)
    Wh, Wl = _pair(W3)
    xT = np.ascontiguousarray(x2.T)            # [F, N]
    adjb = adj2.astype(BF16)                   # exact 0/1

    in_maps = []
    for c in range(NCORES):
        sl = slice(c * SLAB, (c + 1) * SLAB)
        xh, xl = _pair(np.ascontiguousarray(xT[:, sl]))
        in_maps.append({
            "adjT": np.ascontiguousarray(adjb[sl, :].T),
            "xT_hi": xh, "xT_lo": xl,
            "U6": U6, "U3": U3,
            "W_hi": Wh, "W_lo": Wl,
            "Wo_hi": Woh, "Wo_lo": Wol,
        })
    return in_maps


def kernel(x, adj, W_heads, a_heads, W_out, a_out):
    if "nc" not in _CACHE:
        # touch the devices once so any residual bad state from a previous
        # process surfaces (and clears) before the real run
        try:
            import jax
            jax.block_until_ready(jax.numpy.zeros(8))
        except Exception:
            pass
        _CACHE["nc"] = build()
    nc = _CACHE["nc"]
    in_maps = prepare_inputs(x, adj, W_heads, a_heads, W_out, a_out)
    res = run_bass_kernel_spmd(nc, in_maps, list(range(NCORES)))
    out = np.concatenate([res.results[c]["out"] for c in range(NCORES)], axis=0)
    return out.reshape(1, N, NCLS)



# revision 4
# speedup vs baseline: 1.2576x; 1.2576x over previous
"""GAT (2-layer, 3-head) forward on 8 Trainium2 NeuronCores — fp8 edition.

Math: with LeakyReLU slope ALPHA=1.0 the edge score e_ij = s1_i + s2_j is
linear and s1_i cancels inside the row softmax, so each layer reduces to

    h'_i = (sum_j adj_ij * w_j * h_j) / (sum_j adj_ij * w_j),
    w_j = exp(s2_j - C)

i.e. one adjacency matmul against G = [w*h | w].  The adjacency matrix is
0/1 — exact in fp8e4m3 — so both big matmuls run as fp8 DoubleRow
(2 k-tiles per instruction, 2x bf16 throughput) with G carried as an
fp8 hi/lo pair (~8 mantissa bits).  Power-of-2 prescales keep every G
value inside e4m3's ±240 range and cancel in the num/den ratio:
h is computed as x@(16W) and the den columns carry 16w; layer 2 uses
xcat@(64Wo) and 64w2.

Layer 2 softmax stabilizer: each core normalizes its slab with its LOCAL
max s2' and the per-block scale factors exp(C2_b - max_b C2_b) are applied
when combining per-block PSUM partials, so the tiny max-AllGather overlaps
the g2 gather + matmul instead of serializing ahead of them.

Sharding: rows of h' across 8 cores; each core holds DoubleRow-layout
adjacency columns and computes its 512-row slab.  G is built slab-wise
and AllGathered per head so gathers pipeline against compute.
"""
import sys

sys.path.insert(0, "/opt/trn_rl_repo")

import numpy as np
import ml_dtypes

import concourse.bass as bass
import concourse.bacc as bacc
import concourse.mybir as mybir
import concourse.bass_isa as bass_isa
import concourse.tile as tile
from concourse.bass_utils import run_bass_kernel_spmd

BF16 = ml_dtypes.bfloat16
E4 = ml_dtypes.float8_e4m3

N = 4096
F = 768
HID = 768
NH = 3
NCLS = 256
NCORES = 8
SLAB = N // NCORES          # 512 rows per core
NIT = SLAB // 128           # 4 i-tiles per core
NJP = N // 256              # 16 j pair-tiles
NFT = F // 128              # 6 f-tiles
NFP = F // 256              # 3 f pair-tiles
GH = NH * HID               # 2304
NCP = GH // 256             # 9 feature pair-tiles of xcat
SC1 = 16.0                  # h prescale (via 16*W)
SC2 = 64.0                  # h2 prescale (via 64*Wo)
G2P = 264                   # 257 g2 cols padded to 8
DR = mybir.MatmulPerfMode.DoubleRow

AF = mybir.ActivationFunctionType
ALU = mybir.AluOpType


def _enable_ldw_opt():
    # walrus defaults to --enable-ldw-opt=false; with it off every LDWEIGHTS
    # serializes against the previous matmul.  Patch the arg builder so the
    # stationary loads pipeline.
    import concourse.bass_utils as _bu
    if getattr(_bu, "_ldw_opt_patched", False):
        return
    _orig = _bu.get_walrus_args

    def _patched(*a, **k):
        args = _orig(*a, **k)
        return [x.replace("--enable-ldw-opt=false", "--enable-ldw-opt=true")
                for x in args]

    _bu.get_walrus_args = _patched
    _bu._ldw_opt_patched = True


def build():
    dt = mybir.dt
    _enable_ldw_opt()
    nc = bacc.Bacc(num_devices=NCORES)

    adjT_d = nc.dram_tensor("adjT8", [N, SLAB], dt.float8e4, kind="ExternalInput")
    xTh_d = nc.dram_tensor("xT_hi", [F, SLAB], dt.bfloat16, kind="ExternalInput")
    xTl_d = nc.dram_tensor("xT_lo", [F, SLAB], dt.bfloat16, kind="ExternalInput")
    x8h_d = nc.dram_tensor("x8_hi", [F, SLAB], dt.float8e4, kind="ExternalInput")
    x8l_d = nc.dram_tensor("x8_lo", [F, SLAB], dt.float8e4, kind="ExternalInput")
    U6_d = nc.dram_tensor("U6", [F, 8], dt.bfloat16, kind="ExternalInput")
    U3_d = nc.dram_tensor("U3", [F, 8], dt.bfloat16, kind="ExternalInput")
    Wh_d = nc.dram_tensor("W_hi", [NH, F, HID], dt.float8e4, kind="ExternalInput")
    Wl_d = nc.dram_tensor("W_lo", [NH, F, HID], dt.float8e4, kind="ExternalInput")
    Woh_d = nc.dram_tensor("Wo_hi", [GH, NCLS], dt.float8e4, kind="ExternalInput")
    Wol_d = nc.dram_tensor("Wo_lo", [GH, NCLS], dt.float8e4, kind="ExternalInput")
    ao2_d = nc.dram_tensor("ao2", [1, NCLS], dt.float32, kind="ExternalInput")
    out_d = nc.dram_tensor("out", [SLAB, NCLS], dt.float32, kind="ExternalOutput")

    # DRAM scratch + collective buffers.  gs/gf layout:
    # [jp, p, hl, ct, t, f] with j = jp*256 + t*128 + p; head 0 carries an
    # extra ct slot (index 6) holding the den w-columns (16w hi at f0:3,
    # lo at f8:11).
    ncts = [7, 6, 6]
    gs = [nc.dram_tensor(f"gs{h}", [2, 128, 2, ncts[h], 2, 128], dt.float8e4)
          for h in range(NH)]
    gf = [nc.dram_tensor(f"gf{h}", [NJP, 128, 2, ncts[h], 2, 128], dt.float8e4,
                         addr_space="Shared")
          for h in range(NH)]
    s2m_slab = nc.dram_tensor("s2m_slab", [8], dt.float32)
    s2m_full = nc.dram_tensor("s2m_full", [8 * NCORES], dt.float32,
                              addr_space="Shared")
    g2_slab = nc.dram_tensor("g2_slab", [2, 128, 2, 2 * G2P], dt.float8e4)
    g2_full = nc.dram_tensor("g2_full", [NJP, 128, 2, 2 * G2P], dt.float8e4,
                             addr_space="Shared")
    c2_slab = nc.dram_tensor("c2_slab", [1], dt.float32)
    c2_full = nc.dram_tensor("c2_full", [NCORES], dt.float32,
                             addr_space="Shared")

    rg = [list(range(NCORES))]

    with tile.TileContext(nc) as tc:
      with (
          tc.tile_pool(name="adjt", bufs=NJP) as p_adjt,
          tc.tile_pool(name="keep", bufs=1) as p_keep,
      ):
        # ---------------- eager loads ----------------
        xbf_hi, xbf_lo = [], []
        xTh_t = xTh_d.rearrange("(ft p) i -> ft p i", p=128)
        xTl_t = xTl_d.rearrange("(ft p) i -> ft p i", p=128)
        adjt = []
        adjT_t = adjT_d.rearrange("(jp t p) i -> jp p t i", t=2, p=128)

        with (
            tc.tile_pool(name="xw", bufs=1) as p_xw,
            tc.tile_pool(name="small", bufs=1) as p_sm,
            tc.tile_pool(name="gtmp", bufs=1) as p_gt,
        ):
            # s2-path loads first (critical path to the tiny AllGather)
            for ft in range(NFT):
                t = p_xw.tile([128, SLAB], dt.bfloat16, tag="xbf", name="xbf",
                              bufs=12)
                nc.sync.dma_start(t[:], xTh_t[ft])
                xbf_hi.append(t)
                t = p_xw.tile([128, SLAB], dt.bfloat16, tag="xbf", name="xbf",
                              bufs=12)
                nc.scalar.dma_start(t[:], xTl_t[ft])
                xbf_lo.append(t)
            u6 = p_sm.tile([128, NFT, 8], dt.bfloat16, tag="u6", name="u6")
            nc.sync.dma_start(u6[:], U6_d.rearrange("(ft p) c -> p ft c", p=128))
            u3 = p_sm.tile([128, NFT, 8], dt.bfloat16, tag="u3", name="u3")
            nc.scalar.dma_start(u3[:], U3_d.rearrange("(ft p) c -> p ft c", p=128))

            # fp8 x pair (projection stationaries)
            x8h, x8l = [], []
            x8h_t = x8h_d.rearrange("(fp t p) i -> fp p t i", t=2, p=128)
            x8l_t = x8l_d.rearrange("(fp t p) i -> fp p t i", t=2, p=128)
            for fp in range(NFP):
                t = p_xw.tile([128, 2, SLAB], dt.float8e4, tag="x8", name="x8",
                              bufs=2 * NFP)
                nc.sync.dma_start(t[:], x8h_t[fp])
                x8h.append(t)
                t = p_xw.tile([128, 2, SLAB], dt.float8e4, tag="x8", name="x8",
                              bufs=2 * NFP)
                nc.scalar.dma_start(t[:], x8l_t[fp])
                x8l.append(t)

            # fp8 W pair (projection movers)
            whi = [[None] * NFP for _ in range(NH)]
            wlo = [[None] * NFP for _ in range(NH)]
            Wh_t = Wh_d.rearrange("h (fp t p) o -> h fp p t o", t=2, p=128)
            Wl_t = Wl_d.rearrange("h (fp t p) o -> h fp p t o", t=2, p=128)
            for h in range(NH):
                for fp in range(NFP):
                    t = p_xw.tile([128, 2, HID], dt.float8e4, tag="w8",
                                  name="w8", bufs=2 * NH * NFP)
                    nc.sync.dma_start(t[:], Wh_t[h, fp])
                    whi[h][fp] = t
                    t = p_xw.tile([128, 2, HID], dt.float8e4, tag="w8",
                                  name="w8", bufs=2 * NH * NFP)
                    nc.scalar.dma_start(t[:], Wl_t[h, fp])
                    wlo[h][fp] = t

            # adjacency (used by both layers)
            for jp in range(NJP):
                t = p_adjt.tile([128, 2, SLAB], dt.float8e4, tag="adjt",
                                name="adjt")
                eng = nc.sync if jp % 2 == 0 else nc.scalar
                eng.dma_start(t[:], adjT_t[jp])
                adjt.append(t)

            # layer-2 weights + ao2 (kept in outer pool for phase 4/5)
            woh, wol = [], []
            Woh_t = Woh_d.rearrange("(fp t p) c -> fp p t c", t=2, p=128)
            Wol_t = Wol_d.rearrange("(fp t p) c -> fp p t c", t=2, p=128)
            for fp in range(NCP):
                t = p_keep.tile([128, 2, NCLS], dt.float8e4, tag="wo",
                                name="wo", bufs=2 * NCP)
                nc.sync.dma_start(t[:], Woh_t[fp])
                woh.append(t)
                t = p_keep.tile([128, 2, NCLS], dt.float8e4, tag="wo",
                                name="wo", bufs=2 * NCP)
                nc.scalar.dma_start(t[:], Wol_t[fp])
                wol.append(t)
            ao2row = p_keep.tile([1, NCLS], dt.float32, tag="ao2r", name="ao2r")
            nc.sync.dma_start(ao2row[:], ao2_d[:])
            ao2bc = p_keep.tile([128, NCLS], dt.float32, tag="ao2b", name="ao2b")
            nc.gpsimd.partition_broadcast(ao2bc[:], ao2row[:], channels=128)

            # ---------------- s2 + tiny max AllGather ----------------
            s2_sb = []
            for h in range(NH):
                s2_sb.append(p_sm.tile([128, NIT], dt.float32, tag="s2",
                                       name="s2", bufs=NH))
            with tc.tile_pool(name="psS", bufs=2, space="PSUM") as ps_s:
                for it in range(NIT):
                    p6 = ps_s.tile([128, 8], dt.float32, tag="p6", name="p6",
                                   bufs=2)
                    p3 = ps_s.tile([128, 8], dt.float32, tag="p3", name="p3",
                                   bufs=2)
                    for ft in range(NFT):
                        xh = xbf_hi[ft][:, it * 128:(it + 1) * 128]
                        xl = xbf_lo[ft][:, it * 128:(it + 1) * 128]
                        nc.tensor.matmul(p6[:], xh, u6[:, ft, :],
                                         start=(ft == 0), stop=(ft == NFT - 1))
                        nc.tensor.matmul(p3[:], xl, u3[:, ft, :],
                                         start=(ft == 0), stop=(ft == NFT - 1))
                    t6 = p_sm.tile([128, 8], dt.float32, tag="t6", name="t6",
                                   bufs=2)
                    nc.vector.tensor_copy(t6[:], p6[:])
                    tsum = p_sm.tile([128, NH], dt.float32, tag="tsum",
                                     name="tsum", bufs=2)
                    nc.vector.tensor_tensor(tsum[:], t6[:, 0:2 * NH:2],
                                            t6[:, 1:2 * NH:2], ALU.add)
                    for h in range(NH):
                        nc.vector.tensor_tensor(s2_sb[h][:, it:it + 1],
                                                tsum[:, h:h + 1],
                                                p3[:, h:h + 1], ALU.add)

            sm8 = p_sm.tile([1, 8], dt.float32, tag="sm8", name="sm8")
            nc.vector.memset(sm8[:], 0.0)
            for h in range(NH):
                m1 = p_sm.tile([128, 1], dt.float32, tag="m1", name="m1", bufs=2)
                nc.vector.tensor_reduce(m1[:], s2_sb[h][:],
                                        axis=mybir.AxisListType.X, op=ALU.max)
                m2 = p_sm.tile([128, 1], dt.float32, tag="m2", name="m2", bufs=2)
                nc.gpsimd.partition_all_reduce(m2[:], m1[:], channels=128,
                                               reduce_op=bass_isa.ReduceOp.max)
                nc.vector.tensor_copy(sm8[0:1, h:h + 1], m2[0:1, 0:1])
            nc.sync.dma_start(s2m_slab[:].rearrange("(o a) -> o a", o=1), sm8[:])
            nc.gpsimd.collective_compute(
                "AllGather", ALU.bypass, replica_groups=rg,
                ins=[s2m_slab[:]], outs=[s2m_full[:]])

            # ---------------- h16 = x @ (16W), overlap the AllGather -----
            hts = [[None] * NIT for _ in range(NH)]
            with tc.tile_pool(name="psA", bufs=6, space="PSUM") as ps_a:
                for h in range(NH):
                    for it in range(NIT):
                        psh = ps_a.tile([128, 512], dt.float32, tag="psh",
                                        name="psh", bufs=2)
                        psl = ps_a.tile([128, 256], dt.float32, tag="psl",
                                        name="psl", bufs=2)
                        c0 = c1 = 0
                        for lhs_l, rhs_l in ((x8h, whi[h]), (x8h, wlo[h]),
                                             (x8l, whi[h])):
                            for fp in range(NFP):
                                lhs = lhs_l[fp][:, :, it * 128:(it + 1) * 128]
                                rhs = rhs_l[fp]
                                nc.tensor.matmul(
                                    psh[:], lhs, rhs[:, :, 0:512],
                                    start=(c0 == 0), stop=(c0 == 3 * NFP - 1),
                                    perf_mode=DR)
                                c0 += 1
                                nc.tensor.matmul(
                                    psl[:], lhs, rhs[:, :, 512:HID],
                                    start=(c1 == 0), stop=(c1 == 3 * NFP - 1),
                                    perf_mode=DR)
                                c1 += 1
                        ht = p_gt.tile([128, HID], dt.float32, tag="ht",
                                       name="ht", bufs=NH * NIT)
                        nc.vector.tensor_copy(ht[:, 0:512], psh[:])
                        nc.vector.tensor_copy(ht[:, 512:HID], psl[:])
                        hts[h][it] = ht

            # ---------------- w = exp(s2 - C), G build, gathers ----------
            mload = p_sm.tile([1, 8 * NCORES], dt.float32, tag="mload",
                              name="mload")
            nc.sync.dma_start(mload[:],
                              s2m_full[:].rearrange("(o a) -> o a", o=1))
            negC = p_sm.tile([1, NH], dt.float32, tag="negC", name="negC")
            for h in range(NH):
                nc.vector.tensor_reduce(
                    negC[0:1, h:h + 1], mload[0:1, h::8],
                    axis=mybir.AxisListType.X, op=ALU.max, negate=True)
            negCbc = p_sm.tile([128, NH], dt.float32, tag="negCbc",
                               name="negCbc")
            nc.gpsimd.partition_broadcast(negCbc[:], negC[:], channels=128)

            w_sb = []
            for h in range(NH):
                w = p_sm.tile([128, NIT], dt.float32, tag="wexp", name="wexp",
                              bufs=NH)
                nc.scalar.activation(w[:], s2_sb[h][:], AF.Exp,
                                     bias=negCbc[:, h:h + 1])
                w_sb.append(w)

            # den w-columns: 16w hi/lo packed into gs0 ct slot 6
            for it in range(NIT):
                w16 = p_sm.tile([128, 16], dt.float32, tag="w16", name="w16",
                                bufs=2)
                nc.vector.memset(w16[:], 0.0)
                for h in range(NH):
                    nc.vector.tensor_scalar_mul(w16[:, h:h + 1], w_sb[h][:, it:it + 1],
                                                SC1)
                wq = p_sm.tile([128, 16], dt.float8e4, tag="wq", name="wq",
                               bufs=2)
                nc.vector.tensor_copy(wq[:], w16[:])
                wr = p_sm.tile([128, 16], dt.float32, tag="wr", name="wr",
                               bufs=2)
                nc.vector.tensor_tensor(wr[:], w16[:], wq[:], ALU.subtract)
                wout = p_sm.tile([128, 128], dt.float8e4, tag="wout",
                                 name="wout", bufs=2)
                nc.vector.memset(wout[:], 0.0)
                nc.vector.tensor_copy(wout[:, 0:NH], wq[:, 0:NH])
                nc.vector.tensor_copy(wout[:, 8:8 + NH], wr[:, 0:NH])
                nc.sync.dma_start(gs[0][it // 2, :, 0, 6, it % 2, :],
                                  wout[:])
                zpad = p_sm.tile([128, 128], dt.float8e4, tag="zpad",
                                 name="zpad", bufs=2)
                nc.vector.memset(zpad[:], 0.0)
                nc.scalar.dma_start(gs[0][it // 2, :, 1, 6, it % 2, :],
                                    zpad[:])

            for h in range(NH):
                for it in range(NIT):
                    g = p_gt.tile([128, HID], dt.float32, tag="g", name="g",
                                  bufs=3)
                    nc.vector.tensor_scalar_mul(g[:], hts[h][it][:],
                                                w_sb[h][:, it:it + 1])
                    ghi = p_gt.tile([128, HID], dt.float8e4, tag="ghi",
                                    name="ghi", bufs=3)
                    nc.scalar.activation(ghi[:], g[:], AF.Copy)
                    gr = p_gt.tile([128, HID], dt.float32, tag="gr",
                                   name="gr", bufs=3)
                    nc.vector.tensor_tensor(gr[:], g[:], ghi[:], ALU.subtract)
                    glo = p_gt.tile([128, HID], dt.float8e4, tag="glo",
                                    name="glo", bufs=3)
                    nc.scalar.activation(glo[:], gr[:], AF.Copy)
                    jp, tt = it // 2, it % 2
                    nc.sync.dma_start(
                        gs[h][jp, :, 0, 0:6, tt, :],
                        ghi[:].rearrange("p (c f) -> p c f", c=6))
                    nc.scalar.dma_start(
                        gs[h][jp, :, 1, 0:6, tt, :],
                        glo[:].rearrange("p (c f) -> p c f", c=6))
                nc.gpsimd.collective_compute(
                    "AllGather", ALU.bypass, replica_groups=rg,
                    ins=[gs[h][:]], outs=[gf[h][:]])

        # ---------------- L1 adjacency matmul + epilogue ----------------
        xcph = [None] * NCP
        xcpl = [None] * NCP
        for cp in range(NCP):
            xcph[cp] = p_keep.tile([128, 2, SLAB], dt.float8e4, tag="xcp",
                                   name="xcp", bufs=2 * NCP)
            xcpl[cp] = p_keep.tile([128, 2, SLAB], dt.float8e4, tag="xcp",
                                   name="xcp", bufs=2 * NCP)
        with (
            tc.tile_pool(name="gst", bufs=96) as p_gst,
            tc.tile_pool(name="etmp", bufs=1) as p_et,
            tc.tile_pool(name="ps1", bufs=4, space="PSUM") as ps_1,
        ):
            # den first (needs only gf0 slot 6)
            psd = ps_1.tile([128, 512], dt.float32, tag="ps1", name="ps1")
            gwt = []
            for jp in range(NJP):
                t = p_gst.tile([128, 2, 16], dt.float8e4, tag="gwt",
                               name="gwt", bufs=NJP)
                eng = nc.sync if jp % 2 == 0 else nc.scalar
                eng.dma_start(t[:], gf[0][jp, :, 0, 6, :, 0:16])
                gwt.append(t)
            for jp in range(NJP):
                nc.tensor.matmul(psd[0:8, :], gwt[jp][:, :, 0:8], adjt[jp][:],
                                 start=(jp == 0), stop=False, perf_mode=DR)
                nc.tensor.matmul(psd[0:8, :], gwt[jp][:, :, 8:16], adjt[jp][:],
                                 start=False, stop=(jp == NJP - 1), perf_mode=DR)
            recip3 = p_et.tile([NH, SLAB], dt.float32, tag="recip3",
                               name="recip3")
            nc.vector.reciprocal(recip3[:], psd[0:NH, :])
            rbc = []
            for h in range(NH):
                rrow = p_et.tile([1, SLAB], dt.float32, tag="rrow",
                                 name="rrow", bufs=2)
                nc.sync.dma_start(rrow[:], recip3[h:h + 1, :])
                rb = p_et.tile([128, SLAB], dt.float32, tag="rbc",
                               name="rbc", bufs=NH)
                nc.gpsimd.partition_broadcast(rb[:], rrow[:], channels=128)
                rbc.append(rb)

            for ct in range(NH * NFT):
                h, lct = ct // NFT, ct % NFT
                ps = ps_1.tile([128, 512], dt.float32, tag="ps1", name="ps1")
                for jp in range(NJP):
                    gh8 = p_gst.tile([128, 2, 128], dt.float8e4, tag="gst",
                                     name="gst")
                    gl8 = p_gst.tile([128, 2, 128], dt.float8e4, tag="gst",
                                     name="gst")
                    eng = nc.sync if jp % 2 == 0 else nc.scalar
                    eng2 = nc.scalar if jp % 2 == 0 else nc.sync
                    eng.dma_start(gh8[:], gf[h][jp, :, 0, lct, :, :])
                    eng2.dma_start(gl8[:], gf[h][jp, :, 1, lct, :, :])
                    nc.tensor.matmul(ps[:], gh8[:], adjt[jp][:],
                                     start=(jp == 0), stop=False, perf_mode=DR)
                    nc.tensor.matmul(ps[:], gl8[:], adjt[jp][:],
                                     start=False, stop=(jp == NJP - 1),
                                     perf_mode=DR)
                # xcatT tile = elu(numT/den) and its fp8 pair
                z = p_et.tile([128, SLAB], dt.float32, tag="z", name="z",
                              bufs=2)
                nc.vector.tensor_tensor(z[:], ps[:], rbc[h][:], ALU.mult)
                e = p_et.tile([128, SLAB], dt.float32, tag="e", name="e",
                              bufs=2)
                nc.scalar.activation(e[:], z[:], AF.Exp)
                nc.vector.tensor_scalar(e[:], e[:], 1.0, -1.0, ALU.min, ALU.add)
                xc = p_et.tile([128, SLAB], dt.float32, tag="xc", name="xc",
                               bufs=2)
                nc.vector.scalar_tensor_tensor(xc[:], z[:], 0.0, e[:],
                                               ALU.max, ALU.add)
                cp, sub = ct // 2, ct % 2
                nc.scalar.activation(xcph[cp][:, sub, :], xc[:], AF.Copy)
                r32 = p_et.tile([128, SLAB], dt.float32, tag="r32",
                                name="r32", bufs=2)
                nc.vector.tensor_tensor(r32[:], xc[:], xcph[cp][:, sub, :],
                                        ALU.subtract)
                nc.scalar.activation(xcpl[cp][:, sub, :], r32[:], AF.Copy)

        # ---------------- layer 2 ----------------
        with (
            tc.tile_pool(name="l2a", bufs=1) as p_l2a,
            tc.tile_pool(name="psh2", bufs=4, space="PSUM") as ps_h2,
        ):
            ps2l = [ps_h2.tile([128, NCLS], dt.float32, tag="psh2",
                               name="psh2") for _ in range(NIT)]
            for cp in range(NCP):
                for it in range(NIT):
                    xh = xcph[cp][:, :, it * 128:(it + 1) * 128]
                    xl = xcpl[cp][:, :, it * 128:(it + 1) * 128]
                    nc.tensor.matmul(ps2l[it][:], xh, woh[cp][:],
                                     start=(cp == 0), stop=False, perf_mode=DR)
                    nc.tensor.matmul(ps2l[it][:], xh, wol[cp][:],
                                     start=False, stop=False, perf_mode=DR)
                    nc.tensor.matmul(ps2l[it][:], xl, woh[cp][:],
                                     start=False, stop=(cp == NCP - 1),
                                     perf_mode=DR)
            # s2' = (h2_64 @ ao2)/64 per it; local slab max -> w2
            s2p = p_l2a.tile([128, NIT], dt.float32, tag="s2p", name="s2p")
            for it in range(NIT):
                tmp = p_l2a.tile([128, NCLS], dt.float32, tag="s2t",
                                 name="s2t", bufs=2)
                nc.vector.tensor_tensor(tmp[:], ps2l[it][:], ao2bc[:],
                                        ALU.mult)
                red = p_l2a.tile([128, 1], dt.float32, tag="s2r", name="s2r",
                                 bufs=2)
                nc.vector.tensor_reduce(red[:], tmp[:],
                                        axis=mybir.AxisListType.X, op=ALU.add)
                nc.vector.tensor_scalar_mul(s2p[:, it:it + 1], red[:],
                                            1.0 / SC2)
            sm1 = p_l2a.tile([128, 1], dt.float32, tag="sm1", name="sm1")
            nc.vector.tensor_reduce(sm1[:], s2p[:], axis=mybir.AxisListType.X,
                                    op=ALU.max)
            sm2 = p_l2a.tile([128, 1], dt.float32, tag="sm2", name="sm2")
            nc.gpsimd.partition_all_reduce(sm2[:], sm1[:], channels=128,
                                           reduce_op=bass_isa.ReduceOp.max)
            negC2 = p_l2a.tile([128, 1], dt.float32, tag="negC2", name="negC2")
            nc.vector.tensor_scalar_mul(negC2[:], sm2[:], -1.0)
            nc.sync.dma_start(c2_slab[:].rearrange("(o a) -> o a", o=1),
                              sm2[0:1, 0:1])
            w2all = p_l2a.tile([128, NIT], dt.float32, tag="w2all",
                               name="w2all")
            nc.scalar.activation(w2all[:], s2p[:], AF.Exp, bias=negC2[:])
            # g2 = [w2*h2_64 | 64*w2] fp8 pair
            for it in range(NIT):
                g2 = p_l2a.tile([128, G2P], dt.float32, tag="g2", name="g2",
                                bufs=2)
                nc.vector.memset(g2[:], 0.0)
                nc.vector.tensor_scalar_mul(g2[:, 0:NCLS], ps2l[it][:],
                                            w2all[:, it:it + 1])
                nc.vector.tensor_scalar_mul(g2[:, NCLS:NCLS + 1],
                                            w2all[:, it:it + 1], SC2)
                g2h = p_l2a.tile([128, G2P], dt.float8e4, tag="g2h",
                                 name="g2h", bufs=2)
                nc.scalar.activation(g2h[:], g2[:], AF.Copy)
                g2r = p_l2a.tile([128, G2P], dt.float32, tag="g2r",
                                 name="g2r", bufs=2)
                nc.vector.tensor_tensor(g2r[:], g2[:], g2h[:], ALU.subtract)
                g2l = p_l2a.tile([128, G2P], dt.float8e4, tag="g2l",
                                 name="g2l", bufs=2)
                nc.scalar.activation(g2l[:], g2r[:], AF.Copy)
                nc.sync.dma_start(g2_slab[it // 2, :, it % 2, 0:G2P], g2h[:])
                nc.scalar.dma_start(g2_slab[it // 2, :, it % 2, G2P:2 * G2P],
                                    g2l[:])
            nc.gpsimd.collective_compute(
                "AllGather", ALU.bypass, replica_groups=rg,
                ins=[g2_slab[:]], outs=[g2_full[:]])
            nc.gpsimd.collective_compute(
                "AllGather", ALU.bypass, replica_groups=rg,
                ins=[c2_slab[:]], outs=[c2_full[:]])

            # L2 attention: per-block partials combined with exp(C2_b - max)
            with (
                tc.tile_pool(name="g2t", bufs=NJP) as p_g2t,
                tc.tile_pool(name="fin", bufs=1) as p_f,
                tc.tile_pool(name="ps2", bufs=4, space="PSUM") as ps_2,
            ):
                g2tiles = []
                for jp in range(NJP):
                    t = p_g2t.tile([128, 2, 2 * G2P], dt.float8e4, tag="g2t",
                                   name="g2t")
                    eng = nc.sync if jp % 2 == 0 else nc.scalar
                    eng.dma_start(t[:], g2_full[jp])
                    g2tiles.append(t)
                cload = p_f.tile([1, NCORES], dt.float32, tag="cload",
                                 name="cload")
                nc.sync.dma_start(cload[:],
                                  c2_full[:].rearrange("(o a) -> o a", o=1))
                negmx = p_f.tile([1, 1], dt.float32, tag="negmx", name="negmx")
                nc.vector.tensor_reduce(negmx[:], cload[:],
                                        axis=mybir.AxisListType.X,
                                        op=ALU.max, negate=True)
                srow = p_f.tile([1, NCORES], dt.float32, tag="srow",
                                name="srow")
                nc.scalar.activation(srow[:], cload[:], AF.Exp, bias=negmx[:])
                sbc = p_f.tile([128, NCORES], dt.float32, tag="sbc",
                               name="sbc")
                nc.gpsimd.partition_broadcast(sbc[:], srow[:], channels=128)

                accs = []
                for it in range(NIT):
                    acc = p_f.tile([128, 257], dt.float32, tag="acc",
                                   name="acc", bufs=NIT)
                    accs.append(acc)
                    for b in range(NCORES):
                        psb = ps_2.tile([128, 257], dt.float32, tag="ps2",
                                        name="ps2", bufs=2)
                        cnt = 0
                        for jp in (2 * b, 2 * b + 1):
                            lhs = adjt[jp][:, :, it * 128:(it + 1) * 128]
                            nc.tensor.matmul(psb[:], lhs,
                                             g2tiles[jp][:, :, 0:257],
                                             start=(cnt == 0), stop=False,
                                             perf_mode=DR)
                            cnt += 1
                            nc.tensor.matmul(psb[:], lhs,
                                             g2tiles[jp][:, :, G2P:G2P + 257],
                                             start=False, stop=(cnt == 2),
                                             perf_mode=DR)
                        if b == 0:
                            nc.vector.tensor_scalar_mul(acc[:], psb[:],
                                                        sbc[:, 0:1])
                        else:
                            nc.vector.scalar_tensor_tensor(
                                acc[:], psb[:], sbc[:, b:b + 1], acc[:],
                                ALU.mult, ALU.add)
                for it in range(NIT):
                    acc = accs[it]
                    r2 = p_f.tile([128, 1], dt.float32, tag="r2", name="r2",
                                  bufs=2)
                    nc.vector.reciprocal(r2[:], acc[:, NCLS:NCLS + 1])
                    z = p_f.tile([128, NCLS], dt.float32, tag="z2", name="z2",
                                 bufs=2)
                    nc.vector.tensor_scalar_mul(z[:], acc[:, 0:NCLS], r2[:])
                    e = p_f.tile([128, NCLS], dt.float32, tag="e2", name="e2",
                                 bufs=2)
                    nc.scalar.activation(e[:], z[:], AF.Exp)
                    nc.vector.tensor_scalar(e[:], e[:], 1.0, -1.0, ALU.min,
                                            ALU.add)
                    o = p_f.tile([128, NCLS], dt.float32, tag="o2", name="o2",
                                 bufs=2)
                    nc.vector.scalar_tensor_tensor(o[:], z[:], 0.0, e[:],
                                                   ALU.max, ALU.add)
                    negm = p_f.tile([128, 1], dt.float32, tag="negm",
                                    name="negm", bufs=2)
                    nc.vector.tensor_reduce(negm[:], o[:],
                                            axis=mybir.AxisListType.X,
                                            op=ALU.max, negate=True)
                    t = p_f.tile([128, NCLS], dt.float32, tag="texp",
                                 name="texp", bufs=2)
                    nc.scalar.activation(t[:], o[:], AF.Exp, bias=negm[:])
                    ssum = p_f.tile([128, 1], dt.float32, tag="ssum",
                                    name="ssum", bufs=2)
                    nc.vector.tensor_reduce(ssum[:], t[:],
                                            axis=mybir.AxisListType.X,
                                            op=ALU.add)
                    lg = p_f.tile([128, 1], dt.float32, tag="lg", name="lg",
                                  bufs=2)
                    nc.scalar.activation(lg[:], ssum[:], AF.Ln)
                    fin = p_f.tile([128, NCLS], dt.float32, tag="fin",
                                   name="fin", bufs=2)
                    nc.vector.tensor_scalar(fin[:], o[:], negm[:], lg[:],
                                            ALU.add, ALU.subtract)
                    nc.sync.dma_start(out_d[it * 128:(it + 1) * 128, :],
                                      fin[:])

    nc.finalize()
    return nc


_CACHE = {}


def _pair8(a):
    hi = a.astype(E4)
    lo = (a.astype(np.float32) - hi.astype(np.float32)).astype(E4)
    return hi, lo


def _pairb(a):
    hi = a.astype(BF16)
    lo = (a - hi.astype(np.float32)).astype(BF16)
    return hi, lo


def prepare_inputs(x, adj, W_heads, a_heads, W_out, a_out):
    """Shard + lay out the full inputs for the 8 cores."""
    x2 = np.asarray(x, np.float32)[0]          # [N, F]
    adj2 = np.asarray(adj)[0]                  # [N, N] int32
    W3 = np.asarray(W_heads, np.float32).reshape(NH, F, HID)
    a3 = np.asarray(a_heads, np.float32)       # [NH, 2*HID, 1]
    Wo = np.asarray(W_out, np.float32).reshape(GH, NCLS)
    ao = np.asarray(a_out, np.float32)         # [2*NCLS, 1]

    # fold the edge-score projection into the weights: s2 = x @ (W @ a2)
    u = np.einsum("hfo,ho->hf", W3.astype(np.float64),
                  a3[:, HID:, 0].astype(np.float64)).astype(np.float32)
    u_hi, u_lo = _pairb(u)
    U6 = np.zeros((F, 8), BF16)
    U3 = np.zeros((F, 8), BF16)
    for h in range(NH):
        U6[:, 2 * h] = u_hi[h]
        U6[:, 2 * h + 1] = u_lo[h]
        U3[:, h] = u_hi[h]
    Wh, Wl = _pair8(SC1 * W3)
    Woh, Wol = _pair8(SC2 * Wo)
    ao2 = np.ascontiguousarray(ao[NCLS:, 0]).reshape(1, NCLS)
    xT = np.ascontiguousarray(x2.T)            # [F, N]
    adj8 = adj2.astype(E4)                     # exact 0/1

    in_maps = []
    for c in range(NCORES):
        sl = slice(c * SLAB, (c + 1) * SLAB)
        xs = np.ascontiguousarray(xT[:, sl])
        xbh, xbl = _pairb(xs)
        x8h, x8l = _pair8(xs)
        in_maps.append({
            "adjT8": np.ascontiguousarray(adj8[sl, :].T),
            "xT_hi": xbh, "xT_lo": xbl,
            "x8_hi": x8h, "x8_lo": x8l,
            "U6": U6, "U3": U3,
            "W_hi": Wh, "W_lo": Wl,
            "Wo_hi": Woh, "Wo_lo": Wol,
            "ao2": ao2,
        })
    return in_maps


def kernel(x, adj, W_heads, a_heads, W_out, a_out):
    if "nc" not in _CACHE:
        # touch the devices once so any residual bad state from a previous
        # process surfaces (and clears) before the real run
        try:
            import jax
            jax.block_until_ready(jax.numpy.zeros(8))
        except Exception:
            pass
        _CACHE["nc"] = build()
    nc = _CACHE["nc"]
    in_maps = prepare_inputs(x, adj, W_heads, a_heads, W_out, a_out)
    res = run_bass_kernel_spmd(nc, in_maps, list(range(NCORES)))
    out = np.concatenate([res.results[c]["out"] for c in range(NCORES)], axis=0)
    return out.reshape(1, N, NCLS)


# revision 6
# speedup vs baseline: 1.9605x; 1.5589x over previous
"""GAT (2-layer, 3-head) forward on 8 Trainium2 NeuronCores — fp8 gathers.

Math: with LeakyReLU slope ALPHA=1.0 the edge score e_ij = s1_i + s2_j is
linear and s1_i cancels inside the row softmax, so each layer reduces to

    h'_i = (sum_j adj_ij * w_j * h_j) / (sum_j adj_ij * w_j),
    w_j = exp(s2_j - C)

i.e. one adjacency matmul against G = [w*h | w].  The adjacency matrix is
0/1 — exact in fp8e4m3 — and G is shipped in SINGLE fp8 (measured final
rel err ~2.5e-3 vs the 2e-2 gate), so the big matmuls run as fp8
DoubleRow over j-tile pairs: 2 k-tiles and 2 MAC/PE-cell per cycle,
2x bf16 throughput and half the gather bytes.  Power-of-2 prescales keep
G inside e4m3's +-240 range and cancel in the num/den ratio: h = x@(16W),
den columns carry 16w; layer 2 uses xcat@(64Wo) and 64w2.

Projections stay in single bf16 (x-stationary, weight-moving).

Layer 2 softmax stabilizer: each core normalizes its slab with its LOCAL
max s2', and per-block scale factors exp(C2_b - max_b C2_b) are applied
when combining per-block PSUM partials, so the tiny max-AllGather
overlaps the g2 gather + matmul instead of serializing ahead of them.
"""
import sys

sys.path.insert(0, "/opt/trn_rl_repo")

import numpy as np
import ml_dtypes

import concourse.bass as bass
import concourse.bacc as bacc
import concourse.mybir as mybir
import concourse.bass_isa as bass_isa
import concourse.tile as tile
from concourse.bass_utils import run_bass_kernel_spmd

BF16 = ml_dtypes.bfloat16
E4 = ml_dtypes.float8_e4m3

N = 4096
F = 768
HID = 768
NH = 3
NCLS = 256
NCORES = 8
SLAB = N // NCORES          # 512 rows per core
NIT = SLAB // 128           # 4 i-tiles per core
NJP = N // 256              # 16 j pair-tiles
NFT = F // 128              # 6 f-tiles
GH = NH * HID               # 2304
NCT = GH // 128             # 18 feature col-tiles of xcat
SC1 = 16.0                  # h prescale (via 16*W)
SC2 = 64.0                  # h2 prescale (via 64*Wo)
G2P = 264                   # 257 g2 cols padded to 8
DR = mybir.MatmulPerfMode.DoubleRow

AF = mybir.ActivationFunctionType
ALU = mybir.AluOpType


def build():
    dt = mybir.dt
    nc = bacc.Bacc(num_devices=NCORES)

    adjT_d = nc.dram_tensor("adjT8", [N, SLAB], dt.float8e4, kind="ExternalInput")
    xTh_d = nc.dram_tensor("xT_hi", [F, SLAB], dt.bfloat16, kind="ExternalInput")
    xTl_d = nc.dram_tensor("xT_lo", [F, SLAB], dt.bfloat16, kind="ExternalInput")
    U6_d = nc.dram_tensor("U6", [F, 8], dt.bfloat16, kind="ExternalInput")
    U3_d = nc.dram_tensor("U3", [F, 8], dt.bfloat16, kind="ExternalInput")
    W_d = nc.dram_tensor("W16", [NH, F, HID], dt.bfloat16, kind="ExternalInput")
    Wo_d = nc.dram_tensor("Wo64", [GH, NCLS], dt.bfloat16, kind="ExternalInput")
    ao2_d = nc.dram_tensor("ao2", [1, NCLS], dt.float32, kind="ExternalInput")
    out_d = nc.dram_tensor("out", [SLAB, NCLS], dt.float32, kind="ExternalOutput")

    # DRAM scratch + collective buffers.  gs/gf layout: [jp, p, ct, t, f]
    # with j = jp*256 + t*128 + p; head 0 carries an extra ct slot (6)
    # holding the den w-columns (16w at f0:3).
    ncts = [7, 6, 6]
    gs = [nc.dram_tensor(f"gs{h}", [2, 128, ncts[h], 2, 128], dt.float8e4)
          for h in range(NH)]
    gf = [nc.dram_tensor(f"gf{h}", [NJP, 128, ncts[h], 2, 128], dt.float8e4,
                         addr_space="Shared")
          for h in range(NH)]
    s2m_slab = nc.dram_tensor("s2m_slab", [8], dt.float32)
    s2m_full = nc.dram_tensor("s2m_full", [8 * NCORES], dt.float32,
                              addr_space="Shared")
    g2_slab = nc.dram_tensor("g2_slab", [2, 128, 2, G2P], dt.float8e4)
    g2_full = nc.dram_tensor("g2_full", [NJP, 128, 2, G2P], dt.float8e4,
                             addr_space="Shared")
    c2_slab = nc.dram_tensor("c2_slab", [1], dt.float32)
    c2_full = nc.dram_tensor("c2_full", [NCORES], dt.float32,
                             addr_space="Shared")

    rg = [list(range(NCORES))]

    with tile.TileContext(nc) as tc:
      with (
          tc.tile_pool(name="adjt", bufs=NJP) as p_adjt,
          tc.tile_pool(name="keep", bufs=1) as p_keep,
      ):
        with (
            tc.tile_pool(name="xw", bufs=1) as p_xw,
            tc.tile_pool(name="small", bufs=1) as p_sm,
            tc.tile_pool(name="gtmp", bufs=1) as p_gt,
        ):
            # ---------------- eager loads ----------------
            # s2-path loads first (critical path to the tiny AllGather)
            xbf_hi, xbf_lo = [], []
            xTh_t = xTh_d.rearrange("(ft p) i -> ft p i", p=128)
            xTl_t = xTl_d.rearrange("(ft p) i -> ft p i", p=128)
            for ft in range(NFT):
                t = p_xw.tile([128, SLAB], dt.bfloat16, tag="xbf", name="xbf",
                              bufs=12)
                nc.sync.dma_start(t[:], xTh_t[ft])
                xbf_hi.append(t)
                t = p_xw.tile([128, SLAB], dt.bfloat16, tag="xbf", name="xbf",
                              bufs=12)
                nc.scalar.dma_start(t[:], xTl_t[ft])
                xbf_lo.append(t)
            u6 = p_sm.tile([128, NFT, 8], dt.bfloat16, tag="u6", name="u6")
            nc.sync.dma_start(u6[:], U6_d.rearrange("(ft p) c -> p ft c", p=128))
            u3 = p_sm.tile([128, NFT, 8], dt.bfloat16, tag="u3", name="u3")
            nc.scalar.dma_start(u3[:], U3_d.rearrange("(ft p) c -> p ft c", p=128))

            # projection weights (bf16, prescaled by 16)
            wmv = [[None] * NFT for _ in range(NH)]
            W_t = W_d.rearrange("h (ft p) o -> h ft p o", p=128)
            for h in range(NH):
                for ft in range(NFT):
                    t = p_xw.tile([128, HID], dt.bfloat16, tag="w16",
                                  name="w16", bufs=NH * NFT)
                    eng = nc.sync if ft % 2 == 0 else nc.scalar
                    eng.dma_start(t[:], W_t[h, ft])
                    wmv[h][ft] = t

            # adjacency (both layers; DoubleRow j-pair layout)
            adjt = []
            adjT_t = adjT_d.rearrange("(jp t p) i -> jp p t i", t=2, p=128)
            for jp in range(NJP):
                t = p_adjt.tile([128, 2, SLAB], dt.float8e4, tag="adjt",
                                name="adjt")
                eng = nc.sync if jp % 2 == 0 else nc.scalar
                eng.dma_start(t[:], adjT_t[jp])
                adjt.append(t)

            # layer-2 weights + ao2 (outer pool; used in phase 4/5)
            wo = []
            Wo_t = Wo_d.rearrange("(ct p) c -> ct p c", p=128)
            for ct in range(NCT):
                t = p_keep.tile([128, NCLS], dt.bfloat16, tag="wo", name="wo",
                                bufs=NCT)
                eng = nc.sync if ct % 2 == 0 else nc.scalar
                eng.dma_start(t[:], Wo_t[ct])
                wo.append(t)
            ao2row = p_keep.tile([1, NCLS], dt.float32, tag="ao2r", name="ao2r")
            nc.sync.dma_start(ao2row[:], ao2_d[:])

            # ---------------- s2 + tiny max AllGather ----------------
            s2_sb = []
            for h in range(NH):
                s2_sb.append(p_sm.tile([128, NIT], dt.float32, tag="s2",
                                       name="s2", bufs=NH))
            with tc.tile_pool(name="psS", bufs=2, space="PSUM") as ps_s:
                for it in range(NIT):
                    p6 = ps_s.tile([128, 8], dt.float32, tag="p6", name="p6",
                                   bufs=2)
                    p3 = ps_s.tile([128, 8], dt.float32, tag="p3", name="p3",
                                   bufs=2)
                    for ft in range(NFT):
                        xh = xbf_hi[ft][:, it * 128:(it + 1) * 128]
                        xl = xbf_lo[ft][:, it * 128:(it + 1) * 128]
                        nc.tensor.matmul(p6[:], xh, u6[:, ft, :],
                                         start=(ft == 0), stop=(ft == NFT - 1))
                        nc.tensor.matmul(p3[:], xl, u3[:, ft, :],
                                         start=(ft == 0), stop=(ft == NFT - 1))
                    t6 = p_sm.tile([128, 8], dt.float32, tag="t6", name="t6",
                                   bufs=2)
                    nc.vector.tensor_copy(t6[:], p6[:])
                    tsum = p_sm.tile([128, NH], dt.float32, tag="tsum",
                                     name="tsum", bufs=2)
                    nc.vector.tensor_tensor(tsum[:], t6[:, 0:2 * NH:2],
                                            t6[:, 1:2 * NH:2], ALU.add)
                    for h in range(NH):
                        nc.vector.tensor_tensor(s2_sb[h][:, it:it + 1],
                                                tsum[:, h:h + 1],
                                                p3[:, h:h + 1], ALU.add)

            sm8 = p_sm.tile([1, 8], dt.float32, tag="sm8", name="sm8")
            nc.vector.memset(sm8[:], 0.0)
            for h in range(NH):
                m1 = p_sm.tile([128, 1], dt.float32, tag="m1", name="m1", bufs=2)
                nc.vector.tensor_reduce(m1[:], s2_sb[h][:],
                                        axis=mybir.AxisListType.X, op=ALU.max)
                m2 = p_sm.tile([128, 1], dt.float32, tag="m2", name="m2", bufs=2)
                nc.gpsimd.partition_all_reduce(m2[:], m1[:], channels=128,
                                               reduce_op=bass_isa.ReduceOp.max)
                nc.vector.tensor_copy(sm8[0:1, h:h + 1], m2[0:1, 0:1])
            nc.sync.dma_start(s2m_slab[:].rearrange("(o a) -> o a", o=1), sm8[:])
            nc.gpsimd.collective_compute(
                "AllGather", ALU.bypass, replica_groups=rg,
                ins=[s2m_slab[:]], outs=[s2m_full[:]])
            # ao2 broadcast AFTER the collective trigger so gpsimd reaches
            # the rendezvous without waiting on the eager-DMA queue
            ao2bc = p_keep.tile([128, NCLS], dt.float32, tag="ao2b", name="ao2b")
            nc.gpsimd.partition_broadcast(ao2bc[:], ao2row[:], channels=128)

            # ---------------- h16 = x @ (16W), overlap the AllGather -----
            hts = [[None] * NIT for _ in range(NH)]
            with tc.tile_pool(name="psA", bufs=1, space="PSUM") as ps_a:
                for h in range(NH):
                    for it in range(NIT):
                        psh = ps_a.tile([128, 512], dt.float32, tag="psh",
                                        name="psh", bufs=2)
                        psl = ps_a.tile([128, 256], dt.float32, tag="psl",
                                        name="psl", bufs=2)
                        for ft in range(NFT):
                            xh = xbf_hi[ft][:, it * 128:(it + 1) * 128]
                            nc.tensor.matmul(psh[:], xh, wmv[h][ft][:, 0:512],
                                             start=(ft == 0),
                                             stop=(ft == NFT - 1))
                            nc.tensor.matmul(psl[:], xh, wmv[h][ft][:, 512:HID],
                                             start=(ft == 0),
                                             stop=(ft == NFT - 1))
                        ht = p_gt.tile([128, HID], dt.float32, tag="ht",
                                       name="ht", bufs=NH * NIT)
                        nc.vector.tensor_copy(ht[:, 0:512], psh[:])
                        nc.vector.tensor_copy(ht[:, 512:HID], psl[:])
                        hts[h][it] = ht

            # ---------------- w = exp(s2 - C), G build, gathers ----------
            mload = p_sm.tile([1, 8 * NCORES], dt.float32, tag="mload",
                              name="mload")
            nc.sync.dma_start(mload[:],
                              s2m_full[:].rearrange("(o a) -> o a", o=1))
            negC = p_sm.tile([1, NH], dt.float32, tag="negC", name="negC")
            for h in range(NH):
                nc.vector.tensor_reduce(
                    negC[0:1, h:h + 1], mload[0:1, h::8],
                    axis=mybir.AxisListType.X, op=ALU.max, negate=True)
            negCbc = p_sm.tile([128, NH], dt.float32, tag="negCbc",
                               name="negCbc")
            nc.gpsimd.partition_broadcast(negCbc[:], negC[:], channels=128)

            w_sb = []
            for h in range(NH):
                w = p_sm.tile([128, NIT], dt.float32, tag="wexp", name="wexp",
                              bufs=NH)
                nc.scalar.activation(w[:], s2_sb[h][:], AF.Exp,
                                     bias=negCbc[:, h:h + 1])
                w_sb.append(w)

            # den w-columns: 16w packed into gs0 ct slot 6 (f0:3)
            for it in range(NIT):
                w16 = p_sm.tile([128, 16], dt.float32, tag="w16c", name="w16c",
                                bufs=2)
                nc.vector.memset(w16[:], 0.0)
                for h in range(NH):
                    nc.vector.tensor_scalar_mul(w16[:, h:h + 1],
                                                w_sb[h][:, it:it + 1], SC1)
                wout = p_sm.tile([128, 128], dt.float8e4, tag="wout",
                                 name="wout", bufs=2)
                nc.vector.memset(wout[:], 0.0)
                nc.vector.tensor_copy(wout[:, 0:16], w16[:])
                nc.sync.dma_start(gs[0][it // 2, :, 6, it % 2, :], wout[:])

            for h in range(NH):
                for it in range(NIT):
                    g = p_gt.tile([128, HID], dt.float32, tag="g", name="g",
                                  bufs=3)
                    nc.vector.tensor_scalar_mul(g[:], hts[h][it][:],
                                                w_sb[h][:, it:it + 1])
                    ghi = p_gt.tile([128, HID], dt.float8e4, tag="ghi",
                                    name="ghi", bufs=3)
                    nc.scalar.activation(ghi[:], g[:], AF.Copy)
                    jp, tt = it // 2, it % 2
                    nc.sync.dma_start(
                        gs[h][jp, :, 0:6, tt, :],
                        ghi[:].rearrange("p (c f) -> p c f", c=6))
                nc.gpsimd.collective_compute(
                    "AllGather", ALU.bypass, replica_groups=rg,
                    ins=[gs[h][:]], outs=[gf[h][:]])

        # ---------------- L1 adjacency matmul + epilogue ----------------
        xcpb = [None] * NCT
        for ct in range(NCT):
            xcpb[ct] = p_keep.tile([128, SLAB], dt.bfloat16, tag="xcp",
                                   name="xcp", bufs=NCT)
        with (
            tc.tile_pool(name="gst", bufs=96) as p_gst,
            tc.tile_pool(name="etmp", bufs=1) as p_et,
            tc.tile_pool(name="ps1", bufs=4, space="PSUM") as ps_1,
        ):
            # den first (needs only gf0 slot 6)
            psd = ps_1.tile([128, 512], dt.float32, tag="ps1", name="ps1")
            gwt = []
            for jp in range(NJP):
                t = p_gst.tile([128, 2, 16], dt.float8e4, tag="gwt",
                               name="gwt", bufs=NJP)
                eng = nc.sync if jp % 2 == 0 else nc.scalar
                eng.dma_start(t[:], gf[0][jp, :, 6, :, 0:16])
                gwt.append(t)
            for jp in range(NJP):
                nc.tensor.matmul(psd[0:8, :], gwt[jp][:, :, 0:8], adjt[jp][:],
                                 start=(jp == 0), stop=(jp == NJP - 1),
                                 perf_mode=DR)
            recip3 = p_et.tile([NH, SLAB], dt.float32, tag="recip3",
                               name="recip3")
            nc.vector.reciprocal(recip3[:], psd[0:NH, :])
            rbc = []
            for h in range(NH):
                rrow = p_et.tile([1, SLAB], dt.float32, tag="rrow",
                                 name="rrow", bufs=2)
                nc.sync.dma_start(rrow[:], recip3[h:h + 1, :])
                rb = p_et.tile([128, SLAB], dt.float32, tag="rbc",
                               name="rbc", bufs=NH)
                nc.gpsimd.partition_broadcast(rb[:], rrow[:], channels=128)
                rbc.append(rb)

            for ct in range(NCT):
                h, lct = ct // NFT, ct % NFT
                ps = ps_1.tile([128, 512], dt.float32, tag="ps1", name="ps1")
                for jp in range(NJP):
                    g8 = p_gst.tile([128, 2, 128], dt.float8e4, tag="gst",
                                    name="gst")
                    eng = nc.sync if jp % 2 == 0 else nc.scalar
                    eng.dma_start(g8[:], gf[h][jp, :, lct, :, :])
                    nc.tensor.matmul(ps[:], g8[:], adjt[jp][:],
                                     start=(jp == 0), stop=(jp == NJP - 1),
                                     perf_mode=DR)
                # xcatT tile = elu(numT/den) in single bf16
                z = p_et.tile([128, SLAB], dt.float32, tag="z", name="z",
                              bufs=2)
                nc.vector.tensor_tensor(z[:], ps[:], rbc[h][:], ALU.mult)
                e = p_et.tile([128, SLAB], dt.float32, tag="e", name="e",
                              bufs=2)
                nc.scalar.activation(e[:], z[:], AF.Exp)
                nc.vector.tensor_scalar(e[:], e[:], 1.0, -1.0, ALU.min, ALU.add)
                xc = p_et.tile([128, SLAB], dt.float32, tag="xc", name="xc",
                               bufs=2)
                nc.vector.scalar_tensor_tensor(xc[:], z[:], 0.0, e[:],
                                               ALU.max, ALU.add)
                nc.scalar.activation(xcpb[ct][:], xc[:], AF.Copy)

        # ---------------- layer 2 ----------------
        with (
            tc.tile_pool(name="l2a", bufs=1) as p_l2a,
            tc.tile_pool(name="psh2", bufs=1, space="PSUM") as ps_h2,
        ):
            ps2l = [ps_h2.tile([128, NCLS], dt.float32, tag="psh2",
                               name="psh2", bufs=NIT) for _ in range(NIT)]
            for ct in range(NCT):
                for it in range(NIT):
                    xs = xcpb[ct][:, it * 128:(it + 1) * 128]
                    nc.tensor.matmul(ps2l[it][:], xs, wo[ct][:],
                                     start=(ct == 0), stop=(ct == NCT - 1))
            # s2' = (h2_64 @ ao2)/64 per it; local slab max -> w2
            s2p = p_l2a.tile([128, NIT], dt.float32, tag="s2p", name="s2p")
            for it in range(NIT):
                tmp = p_l2a.tile([128, NCLS], dt.float32, tag="s2t",
                                 name="s2t", bufs=2)
                nc.vector.tensor_tensor(tmp[:], ps2l[it][:], ao2bc[:],
                                        ALU.mult)
                red = p_l2a.tile([128, 1], dt.float32, tag="s2r", name="s2r",
                                 bufs=2)
                nc.vector.tensor_reduce(red[:], tmp[:],
                                        axis=mybir.AxisListType.X, op=ALU.add)
                nc.vector.tensor_scalar_mul(s2p[:, it:it + 1], red[:],
                                            1.0 / SC2)
            sm1 = p_l2a.tile([128, 1], dt.float32, tag="sm1", name="sm1")
            nc.vector.tensor_reduce(sm1[:], s2p[:], axis=mybir.AxisListType.X,
                                    op=ALU.max)
            sm2 = p_l2a.tile([128, 1], dt.float32, tag="sm2", name="sm2")
            nc.gpsimd.partition_all_reduce(sm2[:], sm1[:], channels=128,
                                           reduce_op=bass_isa.ReduceOp.max)
            negC2 = p_l2a.tile([128, 1], dt.float32, tag="negC2", name="negC2")
            nc.vector.tensor_scalar_mul(negC2[:], sm2[:], -1.0)
            nc.sync.dma_start(c2_slab[:].rearrange("(o a) -> o a", o=1),
                              sm2[0:1, 0:1])
            w2all = p_l2a.tile([128, NIT], dt.float32, tag="w2all",
                               name="w2all")
            nc.scalar.activation(w2all[:], s2p[:], AF.Exp, bias=negC2[:])
            # g2 = [w2*h2_64 | 64*w2] single fp8
            for it in range(NIT):
                g2 = p_l2a.tile([128, G2P], dt.float32, tag="g2", name="g2",
                                bufs=2)
                nc.vector.memset(g2[:], 0.0)
                nc.vector.tensor_scalar_mul(g2[:, 0:NCLS], ps2l[it][:],
                                            w2all[:, it:it + 1])
                nc.vector.tensor_scalar_mul(g2[:, NCLS:NCLS + 1],
                                            w2all[:, it:it + 1], SC2)
                g2h = p_l2a.tile([128, G2P], dt.float8e4, tag="g2h",
                                 name="g2h", bufs=2)
                nc.scalar.activation(g2h[:], g2[:], AF.Copy)
                nc.sync.dma_start(g2_slab[it // 2, :, it % 2, :], g2h[:])
            nc.gpsimd.collective_compute(
                "AllGather", ALU.bypass, replica_groups=rg,
                ins=[g2_slab[:]], outs=[g2_full[:]])
            nc.gpsimd.collective_compute(
                "AllGather", ALU.bypass, replica_groups=rg,
                ins=[c2_slab[:]], outs=[c2_full[:]])

            # L2 attention: per-block partials combined with exp(C2_b - max)
            with (
                tc.tile_pool(name="g2t", bufs=NJP) as p_g2t,
                tc.tile_pool(name="fin", bufs=1) as p_f,
                tc.tile_pool(name="ps2", bufs=1, space="PSUM") as ps_2,
            ):
                g2tiles = []
                for jp in range(NJP):
                    t = p_g2t.tile([128, 2, G2P], dt.float8e4, tag="g2t",
                                   name="g2t")
                    eng = nc.sync if jp % 2 == 0 else nc.scalar
                    eng.dma_start(t[:], g2_full[jp])
                    g2tiles.append(t)
                cload = p_f.tile([1, NCORES], dt.float32, tag="cload",
                                 name="cload")
                nc.sync.dma_start(cload[:],
                                  c2_full[:].rearrange("(o a) -> o a", o=1))
                negmx = p_f.tile([1, 1], dt.float32, tag="negmx", name="negmx")
                nc.vector.tensor_reduce(negmx[:], cload[:],
                                        axis=mybir.AxisListType.X,
                                        op=ALU.max, negate=True)
                srow = p_f.tile([1, NCORES], dt.float32, tag="srow",
                                name="srow")
                nc.scalar.activation(srow[:], cload[:], AF.Exp, bias=negmx[:])
                sbc = p_f.tile([128, NCORES], dt.float32, tag="sbc",
                               name="sbc")
                nc.gpsimd.partition_broadcast(sbc[:], srow[:], channels=128)

                accs = []
                for it in range(NIT):
                    acc = p_f.tile([128, G2P], dt.float32, tag="acc",
                                   name="acc", bufs=NIT)
                    accs.append(acc)
                    for b in range(NCORES):
                        psb = ps_2.tile([128, G2P], dt.float32, tag="ps2",
                                        name="ps2", bufs=2)
                        for k, jp in enumerate((2 * b, 2 * b + 1)):
                            lhs = adjt[jp][:, :, it * 128:(it + 1) * 128]
                            nc.tensor.matmul(psb[:], lhs, g2tiles[jp][:],
                                             start=(k == 0), stop=(k == 1),
                                             perf_mode=DR)
                        if b == 0:
                            nc.vector.tensor_scalar_mul(acc[:], psb[:],
                                                        sbc[:, 0:1])
                        else:
                            nc.vector.scalar_tensor_tensor(
                                acc[:], psb[:], sbc[:, b:b + 1], acc[:],
                                ALU.mult, ALU.add)
                for it in range(NIT):
                    acc = accs[it]
                    r2 = p_f.tile([128, 1], dt.float32, tag="r2", name="r2",
                                  bufs=2)
                    nc.vector.reciprocal(r2[:], acc[:, NCLS:NCLS + 1])
                    z = p_f.tile([128, NCLS], dt.float32, tag="z2", name="z2",
                                 bufs=2)
                    nc.vector.tensor_scalar_mul(z[:], acc[:, 0:NCLS], r2[:])
                    e = p_f.tile([128, NCLS], dt.float32, tag="e2", name="e2",
                                 bufs=2)
                    nc.scalar.activation(e[:], z[:], AF.Exp)
                    nc.vector.tensor_scalar(e[:], e[:], 1.0, -1.0, ALU.min,
                                            ALU.add)
                    o = p_f.tile([128, NCLS], dt.float32, tag="o2", name="o2",
                                 bufs=2)
                    nc.vector.scalar_tensor_tensor(o[:], z[:], 0.0, e[:],
                                                   ALU.max, ALU.add)
                    negm = p_f.tile([128, 1], dt.float32, tag="negm",
                                    name="negm", bufs=2)
                    nc.vector.tensor_reduce(negm[:], o[:],
                                            axis=mybir.AxisListType.X,
                                            op=ALU.max, negate=True)
                    t = p_f.tile([128, NCLS], dt.float32, tag="texp",
                                 name="texp", bufs=2)
                    nc.scalar.activation(t[:], o[:], AF.Exp, bias=negm[:])
                    ssum = p_f.tile([128, 1], dt.float32, tag="ssum",
                                    name="ssum", bufs=2)
                    nc.vector.tensor_reduce(ssum[:], t[:],
                                            axis=mybir.AxisListType.X,
                                            op=ALU.add)
                    lg = p_f.tile([128, 1], dt.float32, tag="lg", name="lg",
                                  bufs=2)
                    nc.scalar.activation(lg[:], ssum[:], AF.Ln)
                    fin = p_f.tile([128, NCLS], dt.float32, tag="fin",
                                   name="fin", bufs=2)
                    nc.vector.tensor_scalar(fin[:], o[:], negm[:], lg[:],
                                            ALU.add, ALU.subtract)
                    nc.sync.dma_start(out_d[it * 128:(it + 1) * 128, :],
                                      fin[:])

    nc.finalize()
    return nc


_CACHE = {}


def _pairb(a):
    hi = a.astype(BF16)
    lo = (a - hi.astype(np.float32)).astype(BF16)
    return hi, lo


def prepare_inputs(x, adj, W_heads, a_heads, W_out, a_out):
    """Shard + lay out the full inputs for the 8 cores."""
    x2 = np.asarray(x, np.float32)[0]          # [N, F]
    adj2 = np.asarray(adj)[0]                  # [N, N] int32
    W3 = np.asarray(W_heads, np.float32).reshape(NH, F, HID)
    a3 = np.asarray(a_heads, np.float32)       # [NH, 2*HID, 1]
    Wo = np.asarray(W_out, np.float32).reshape(GH, NCLS)
    ao = np.asarray(a_out, np.float32)         # [2*NCLS, 1]

    # fold the edge-score projection into the weights: s2 = x @ (W @ a2)
    u = np.einsum("hfo,ho->hf", W3.astype(np.float64),
                  a3[:, HID:, 0].astype(np.float64)).astype(np.float32)
    u_hi, u_lo = _pairb(u)
    U6 = np.zeros((F, 8), BF16)
    U3 = np.zeros((F, 8), BF16)
    for h in range(NH):
        U6[:, 2 * h] = u_hi[h]
        U6[:, 2 * h + 1] = u_lo[h]
        U3[:, h] = u_hi[h]
    W16 = (SC1 * W3).astype(BF16)
    Wo64 = (SC2 * Wo).astype(BF16)
    ao2 = np.ascontiguousarray(ao[NCLS:, 0]).reshape(1, NCLS)
    xT = np.ascontiguousarray(x2.T)            # [F, N]
    adj8 = adj2.astype(E4)                     # exact 0/1

    in_maps = []
    for c in range(NCORES):
        sl = slice(c * SLAB, (c + 1) * SLAB)
        xbh, xbl = _pairb(np.ascontiguousarray(xT[:, sl]))
        in_maps.append({
            "adjT8": np.ascontiguousarray(adj8[sl, :].T),
            "xT_hi": xbh, "xT_lo": xbl,
            "U6": U6, "U3": U3,
            "W16": W16, "Wo64": Wo64,
            "ao2": ao2,
        })
    return in_maps


def kernel(x, adj, W_heads, a_heads, W_out, a_out):
    if "nc" not in _CACHE:
        # touch the devices once so any residual bad state from a previous
        # process surfaces (and clears) before the real run
        try:
            import jax
            jax.block_until_ready(jax.numpy.zeros(8))
        except Exception:
            pass
        _CACHE["nc"] = build()
    nc = _CACHE["nc"]
    in_maps = prepare_inputs(x, adj, W_heads, a_heads, W_out, a_out)
    res = run_bass_kernel_spmd(nc, in_maps, list(range(NCORES)))
    out = np.concatenate([res.results[c]["out"] for c in range(NCORES)], axis=0)
    return out.reshape(1, N, NCLS)


# revision 14
# speedup vs baseline: 1.9883x; 1.0142x over previous
"""GAT (2-layer, 3-head) forward on 8 Trainium2 NeuronCores — fp8 gathers.

Math: with LeakyReLU slope ALPHA=1.0 the edge score e_ij = s1_i + s2_j is
linear and s1_i cancels inside the row softmax, so each layer reduces to

    h'_i = (sum_j adj_ij * w_j * h_j) / (sum_j adj_ij * w_j),
    w_j = exp(s2_j - C)

i.e. one adjacency matmul against G = [w*h | w].  The adjacency matrix is
0/1 — exact in fp8e4m3 — and G is shipped in SINGLE fp8 (measured final
rel err ~2.5e-3 vs the 2e-2 gate), so the big matmuls run as fp8
DoubleRow over j-tile pairs: 2 k-tiles and 2 MAC/PE-cell per cycle,
2x bf16 throughput and half the gather bytes.  Power-of-2 prescales keep
G inside e4m3's +-240 range and cancel in the num/den ratio: h = x@(16W),
den columns carry 16w; layer 2 uses xcat@(64Wo) and 64w2.

Projections stay in single bf16 (x-stationary, weight-moving).

Layer 2 softmax stabilizer: each core normalizes its slab with its LOCAL
max s2', and per-block scale factors exp(C2_b - max_b C2_b) are applied
when combining per-block PSUM partials, so the tiny max-AllGather
overlaps the g2 gather + matmul instead of serializing ahead of them.
"""
import sys

sys.path.insert(0, "/opt/trn_rl_repo")

import numpy as np
import ml_dtypes

import concourse.bass as bass
import concourse.bacc as bacc
import concourse.mybir as mybir
import concourse.bass_isa as bass_isa
import concourse.tile as tile
from concourse.bass_utils import run_bass_kernel_spmd

BF16 = ml_dtypes.bfloat16
E4 = ml_dtypes.float8_e4m3

N = 4096
F = 768
HID = 768
NH = 3
NCLS = 256
NCORES = 8
SLAB = N // NCORES          # 512 rows per core
NIT = SLAB // 128           # 4 i-tiles per core
NJP = N // 256              # 16 j pair-tiles
NFT = F // 128              # 6 f-tiles
GH = NH * HID               # 2304
NCT = GH // 128             # 18 feature col-tiles of xcat
SC1 = 16.0                  # h prescale (via 16*W)
SC2 = 64.0                  # h2 prescale (via 64*Wo)
G2P = 264                   # 257 g2 cols padded to 8
DR = mybir.MatmulPerfMode.DoubleRow

AF = mybir.ActivationFunctionType
ALU = mybir.AluOpType


def build():
    dt = mybir.dt
    nc = bacc.Bacc(num_devices=NCORES)

    adjT_d = nc.dram_tensor("adjT8", [N, SLAB], dt.float8e4, kind="ExternalInput")
    xTh_d = nc.dram_tensor("xT_hi", [F, SLAB], dt.bfloat16, kind="ExternalInput")
    xTl_d = nc.dram_tensor("xT_lo", [F, SLAB], dt.bfloat16, kind="ExternalInput")
    U6_d = nc.dram_tensor("U6", [F, 8], dt.bfloat16, kind="ExternalInput")
    U3_d = nc.dram_tensor("U3", [F, 8], dt.bfloat16, kind="ExternalInput")
    W_d = nc.dram_tensor("W16", [NH, F, HID], dt.bfloat16, kind="ExternalInput")
    Wo_d = nc.dram_tensor("Wo64", [GH, NCLS], dt.bfloat16, kind="ExternalInput")
    ao2_d = nc.dram_tensor("ao2", [1, NCLS], dt.float32, kind="ExternalInput")
    out_d = nc.dram_tensor("out", [SLAB, NCLS], dt.float32, kind="ExternalOutput")

    # DRAM scratch + collective buffers.  gs/gf layout: [jp, p, ct, t, f]
    # with j = jp*256 + t*128 + p; head 0 carries an extra ct slot (6)
    # holding the den w-columns (16w at f0:3).
    ncts = [7, 6, 6]
    gs = [nc.dram_tensor(f"gs{h}", [2, 128, ncts[h], 2, 128], dt.float8e4)
          for h in range(NH)]
    gf = [nc.dram_tensor(f"gf{h}", [NJP, 128, ncts[h], 2, 128], dt.float8e4,
                         addr_space="Shared")
          for h in range(NH)]
    s2m_slab = nc.dram_tensor("s2m_slab", [8], dt.float32)
    s2m_full = nc.dram_tensor("s2m_full", [8 * NCORES], dt.float32,
                              addr_space="Shared")
    g2_slab = nc.dram_tensor("g2_slab", [2, 128, 2, G2P], dt.float8e4)
    g2_full = nc.dram_tensor("g2_full", [NJP, 128, 2, G2P], dt.float8e4,
                             addr_space="Shared")
    c2_slab = nc.dram_tensor("c2_slab", [1], dt.float32)
    c2_full = nc.dram_tensor("c2_full", [NCORES], dt.float32,
                             addr_space="Shared")

    rg = [list(range(NCORES))]

    with tile.TileContext(nc) as tc:
      with (
          tc.tile_pool(name="adjt", bufs=NJP) as p_adjt,
          tc.tile_pool(name="keep", bufs=1) as p_keep,
      ):
        with (
            tc.tile_pool(name="xw", bufs=1) as p_xw,
            tc.tile_pool(name="small", bufs=1) as p_sm,
            tc.tile_pool(name="gtmp", bufs=1) as p_gt,
        ):
            # ---------------- eager loads ----------------
            # s2-path loads first (critical path to the tiny AllGather)
            xbf_hi, xbf_lo = [], []
            xTh_t = xTh_d.rearrange("(ft p) i -> ft p i", p=128)
            xTl_t = xTl_d.rearrange("(ft p) i -> ft p i", p=128)
            for ft in range(NFT):
                t = p_xw.tile([128, SLAB], dt.bfloat16, tag="xbf", name="xbf",
                              bufs=12)
                nc.sync.dma_start(t[:], xTh_t[ft])
                xbf_hi.append(t)
                t = p_xw.tile([128, SLAB], dt.bfloat16, tag="xbf", name="xbf",
                              bufs=12)
                nc.scalar.dma_start(t[:], xTl_t[ft])
                xbf_lo.append(t)
            u6 = p_sm.tile([128, NFT, 8], dt.bfloat16, tag="u6", name="u6")
            nc.sync.dma_start(u6[:], U6_d.rearrange("(ft p) c -> p ft c", p=128))
            u3 = p_sm.tile([128, NFT, 8], dt.bfloat16, tag="u3", name="u3")
            nc.scalar.dma_start(u3[:], U3_d.rearrange("(ft p) c -> p ft c", p=128))

            # projection weights (bf16, prescaled by 16)
            wmv = [[None] * NFT for _ in range(NH)]
            W_t = W_d.rearrange("h (ft p) o -> h ft p o", p=128)
            for h in range(NH):
                for ft in range(NFT):
                    t = p_xw.tile([128, HID], dt.bfloat16, tag="w16",
                                  name="w16", bufs=NH * NFT)
                    eng = nc.sync if ft % 2 == 0 else nc.scalar
                    eng.dma_start(t[:], W_t[h, ft])
                    wmv[h][ft] = t

            # adjacency (both layers; DoubleRow j-pair layout)
            adjt = []
            adjT_t = adjT_d.rearrange("(jp t p) i -> jp p t i", t=2, p=128)
            for jp in range(NJP):
                t = p_adjt.tile([128, 2, SLAB], dt.float8e4, tag="adjt",
                                name="adjt")
                eng = nc.sync if jp % 2 == 0 else nc.scalar
                eng.dma_start(t[:], adjT_t[jp])
                adjt.append(t)

            # layer-2 weights + ao2 (outer pool; used in phase 4/5)
            wo = []
            Wo_t = Wo_d.rearrange("(ct p) c -> ct p c", p=128)
            for ct in range(NCT):
                t = p_keep.tile([128, NCLS], dt.bfloat16, tag="wo", name="wo",
                                bufs=NCT)
                eng = nc.sync if ct % 2 == 0 else nc.scalar
                eng.dma_start(t[:], Wo_t[ct])
                wo.append(t)
            ao2row = p_keep.tile([1, NCLS], dt.float32, tag="ao2r", name="ao2r")
            nc.sync.dma_start(ao2row[:], ao2_d[:])

            # ---------------- s2 + tiny max AllGather ----------------
            s2_sb = []
            for h in range(NH):
                s2_sb.append(p_sm.tile([128, NIT], dt.float32, tag="s2",
                                       name="s2", bufs=NH))
            with tc.tile_pool(name="psS", bufs=2, space="PSUM") as ps_s:
                for it in range(NIT):
                    p6 = ps_s.tile([128, 8], dt.float32, tag="p6", name="p6",
                                   bufs=2)
                    p3 = ps_s.tile([128, 8], dt.float32, tag="p3", name="p3",
                                   bufs=2)
                    for ft in range(NFT):
                        xh = xbf_hi[ft][:, it * 128:(it + 1) * 128]
                        xl = xbf_lo[ft][:, it * 128:(it + 1) * 128]
                        nc.tensor.matmul(p6[:], xh, u6[:, ft, :],
                                         start=(ft == 0), stop=(ft == NFT - 1))
                        nc.tensor.matmul(p3[:], xl, u3[:, ft, :],
                                         start=(ft == 0), stop=(ft == NFT - 1))
                    t6 = p_sm.tile([128, 8], dt.float32, tag="t6", name="t6",
                                   bufs=2)
                    nc.vector.tensor_copy(t6[:], p6[:])
                    tsum = p_sm.tile([128, NH], dt.float32, tag="tsum",
                                     name="tsum", bufs=2)
                    nc.vector.tensor_tensor(tsum[:], t6[:, 0:2 * NH:2],
                                            t6[:, 1:2 * NH:2], ALU.add)
                    for h in range(NH):
                        nc.vector.tensor_tensor(s2_sb[h][:, it:it + 1],
                                                tsum[:, h:h + 1],
                                                p3[:, h:h + 1], ALU.add)

            sm8 = p_sm.tile([1, 8], dt.float32, tag="sm8", name="sm8")
            nc.vector.memset(sm8[:], 0.0)
            for h in range(NH):
                m1 = p_sm.tile([128, 1], dt.float32, tag="m1", name="m1", bufs=2)
                nc.vector.tensor_reduce(m1[:], s2_sb[h][:],
                                        axis=mybir.AxisListType.X, op=ALU.max)
                m2 = p_sm.tile([128, 1], dt.float32, tag="m2", name="m2", bufs=2)
                nc.gpsimd.partition_all_reduce(m2[:], m1[:], channels=128,
                                               reduce_op=bass_isa.ReduceOp.max)
                nc.vector.tensor_copy(sm8[0:1, h:h + 1], m2[0:1, 0:1])
            nc.sync.dma_start(s2m_slab[:].rearrange("(o a) -> o a", o=1), sm8[:])
            nc.gpsimd.collective_compute(
                "AllGather", ALU.bypass, replica_groups=rg,
                ins=[s2m_slab[:]], outs=[s2m_full[:]])
            # ao2 broadcast AFTER the collective trigger so gpsimd reaches
            # the rendezvous without waiting on the eager-DMA queue
            ao2bc = p_keep.tile([128, NCLS], dt.float32, tag="ao2b", name="ao2b")
            nc.gpsimd.partition_broadcast(ao2bc[:], ao2row[:], channels=128)

            # ---------------- h16 = x @ (16W), overlap the AllGather -----
            hts = [[None] * NIT for _ in range(NH)]
            with tc.tile_pool(name="psA", bufs=1, space="PSUM") as ps_a:
                for h in range(NH):
                    for it in range(NIT):
                        psh = ps_a.tile([128, 512], dt.float32, tag="psh",
                                        name="psh", bufs=2)
                        psl = ps_a.tile([128, 256], dt.float32, tag="psl",
                                        name="psl", bufs=2)
                        for ft in range(NFT):
                            xh = xbf_hi[ft][:, it * 128:(it + 1) * 128]
                            nc.tensor.matmul(psh[:], xh, wmv[h][ft][:, 0:512],
                                             start=(ft == 0),
                                             stop=(ft == NFT - 1))
                            nc.tensor.matmul(psl[:], xh, wmv[h][ft][:, 512:HID],
                                             start=(ft == 0),
                                             stop=(ft == NFT - 1))
                        ht = p_gt.tile([128, HID], dt.float32, tag="ht",
                                       name="ht", bufs=NH * NIT)
                        nc.vector.tensor_copy(ht[:, 0:512], psh[:])
                        nc.vector.tensor_copy(ht[:, 512:HID], psl[:])
                        hts[h][it] = ht

            # ---------------- w = exp(s2 - C), G build, gathers ----------
            mload = p_sm.tile([1, 8 * NCORES], dt.float32, tag="mload",
                              name="mload")
            nc.sync.dma_start(mload[:],
                              s2m_full[:].rearrange("(o a) -> o a", o=1))
            negC = p_sm.tile([1, NH], dt.float32, tag="negC", name="negC")
            for h in range(NH):
                nc.vector.tensor_reduce(
                    negC[0:1, h:h + 1], mload[0:1, h::8],
                    axis=mybir.AxisListType.X, op=ALU.max, negate=True)
            negCbc = p_sm.tile([128, NH], dt.float32, tag="negCbc",
                               name="negCbc")
            nc.gpsimd.partition_broadcast(negCbc[:], negC[:], channels=128)

            w_sb = []
            for h in range(NH):
                w = p_sm.tile([128, NIT], dt.float32, tag="wexp", name="wexp",
                              bufs=NH)
                nc.scalar.activation(w[:], s2_sb[h][:], AF.Exp,
                                     bias=negCbc[:, h:h + 1])
                w_sb.append(w)

            # den w-columns: 16w packed into gs0 ct slot 6 (f0:3)
            for it in range(NIT):
                w16 = p_sm.tile([128, 16], dt.float32, tag="w16c", name="w16c",
                                bufs=2)
                nc.vector.memset(w16[:], 0.0)
                for h in range(NH):
                    nc.vector.tensor_scalar_mul(w16[:, h:h + 1],
                                                w_sb[h][:, it:it + 1], SC1)
                wout = p_sm.tile([128, 128], dt.float8e4, tag="wout",
                                 name="wout", bufs=2)
                nc.vector.memset(wout[:], 0.0)
                nc.vector.tensor_copy(wout[:, 0:16], w16[:])
                nc.sync.dma_start(gs[0][it // 2, :, 6, it % 2, :], wout[:])

            for h in range(NH):
                for it in range(NIT):
                    g = p_gt.tile([128, HID], dt.float32, tag="g", name="g",
                                  bufs=3)
                    nc.vector.tensor_scalar_mul(g[:], hts[h][it][:],
                                                w_sb[h][:, it:it + 1])
                    ghi = p_gt.tile([128, HID], dt.float8e4, tag="ghi",
                                    name="ghi", bufs=3)
                    nc.gpsimd.tensor_copy(ghi[:], g[:])
                    jp, tt = it // 2, it % 2
                    nc.sync.dma_start(
                        gs[h][jp, :, 0:6, tt, :],
                        ghi[:].rearrange("p (c f) -> p c f", c=6))
                nc.gpsimd.collective_compute(
                    "AllGather", ALU.bypass, replica_groups=rg,
                    ins=[gs[h][:]], outs=[gf[h][:]])

        # ---------------- L1 adjacency matmul + epilogue ----------------
        xcpb = [None] * NCT
        for ct in range(NCT):
            xcpb[ct] = p_keep.tile([128, SLAB], dt.bfloat16, tag="xcp",
                                   name="xcp", bufs=NCT)
        with (
            tc.tile_pool(name="gst", bufs=96) as p_gst,
            tc.tile_pool(name="etmp", bufs=1) as p_et,
            tc.tile_pool(name="ps1", bufs=4, space="PSUM") as ps_1,
        ):
            # den first (needs only gf0 slot 6)
            psd = ps_1.tile([128, 512], dt.float32, tag="ps1", name="ps1")
            gwt = p_gst.tile([128, NJP, 2, 16], dt.float8e4, tag="gwt",
                             name="gwt", bufs=1)
            gf0v = gf[0].rearrange("jp p c t f -> p jp c t f")
            for tt in range(2):
                nc.sync.dma_start(gwt[:, :, tt, :], gf0v[:, :, 6, tt, 0:16])
            for jp in range(NJP):
                nc.tensor.matmul(psd[0:8, :], gwt[:, jp, :, 0:8], adjt[jp][:],
                                 start=(jp == 0), stop=(jp == NJP - 1),
                                 perf_mode=DR)
            recip3 = p_et.tile([NH, SLAB], dt.float32, tag="recip3",
                               name="recip3")
            nc.vector.reciprocal(recip3[:], psd[0:NH, :])
            rbc = []
            for h in range(NH):
                rrow = p_et.tile([1, SLAB], dt.float32, tag="rrow",
                                 name="rrow", bufs=2)
                nc.sync.dma_start(rrow[:], recip3[h:h + 1, :])
                rb = p_et.tile([128, SLAB], dt.float32, tag="rbc",
                               name="rbc", bufs=NH)
                nc.gpsimd.partition_broadcast(rb[:], rrow[:], channels=128)
                rbc.append(rb)

            for ct in range(NCT):
                h, lct = ct // NFT, ct % NFT
                ps = ps_1.tile([128, 512], dt.float32, tag="ps1", name="ps1")
                g8 = p_gst.tile([128, NJP, 2, 128], dt.float8e4, tag="gst",
                                name="gst", bufs=4)
                gv = gf[h].rearrange("jp p c t f -> p jp c t f")
                for tt in range(2):
                    eng = nc.sync if (ct + tt) % 2 == 0 else nc.gpsimd
                    eng.dma_start(g8[:, :, tt, :], gv[:, :, lct, tt, :])
                for jp in range(NJP):
                    nc.tensor.matmul(ps[:], g8[:, jp, :, :], adjt[jp][:],
                                     start=(jp == 0), stop=(jp == NJP - 1),
                                     perf_mode=DR)
                # xcatT tile = elu(numT/den) in single bf16
                z = p_et.tile([128, SLAB], dt.float32, tag="z", name="z",
                              bufs=2)
                nc.vector.tensor_tensor(z[:], ps[:], rbc[h][:], ALU.mult)
                e = p_et.tile([128, SLAB], dt.float32, tag="e", name="e",
                              bufs=2)
                nc.scalar.activation(e[:], z[:], AF.Exp)
                nc.vector.tensor_scalar(e[:], e[:], 1.0, -1.0, ALU.min, ALU.add)
                xc = p_et.tile([128, SLAB], dt.float32, tag="xc", name="xc",
                               bufs=2)
                nc.vector.scalar_tensor_tensor(xc[:], z[:], 0.0, e[:],
                                               ALU.max, ALU.add)
                nc.gpsimd.tensor_copy(xcpb[ct][:], xc[:])

        # ---------------- layer 2 ----------------
        with (
            tc.tile_pool(name="l2a", bufs=1) as p_l2a,
            tc.tile_pool(name="psh2", bufs=1, space="PSUM") as ps_h2,
        ):
            ps2l = [ps_h2.tile([128, NCLS], dt.float32, tag="psh2",
                               name="psh2", bufs=NIT) for _ in range(NIT)]
            for ct in range(NCT):
                for it in range(NIT):
                    xs = xcpb[ct][:, it * 128:(it + 1) * 128]
                    nc.tensor.matmul(ps2l[it][:], xs, wo[ct][:],
                                     start=(ct == 0), stop=(ct == NCT - 1))
            # s2' = (h2_64 @ ao2)/64 per it; local slab max -> w2
            s2p = p_l2a.tile([128, NIT], dt.float32, tag="s2p", name="s2p")
            for it in range(NIT):
                tmp = p_l2a.tile([128, NCLS], dt.float32, tag="s2t",
                                 name="s2t", bufs=2)
                nc.vector.tensor_tensor(tmp[:], ps2l[it][:], ao2bc[:],
                                        ALU.mult)
                red = p_l2a.tile([128, 1], dt.float32, tag="s2r", name="s2r",
                                 bufs=2)
                nc.vector.tensor_reduce(red[:], tmp[:],
                                        axis=mybir.AxisListType.X, op=ALU.add)
                nc.vector.tensor_scalar_mul(s2p[:, it:it + 1], red[:],
                                            1.0 / SC2)
            sm1 = p_l2a.tile([128, 1], dt.float32, tag="sm1", name="sm1")
            nc.vector.tensor_reduce(sm1[:], s2p[:], axis=mybir.AxisListType.X,
                                    op=ALU.max)
            sm2 = p_l2a.tile([128, 1], dt.float32, tag="sm2", name="sm2")
            nc.gpsimd.partition_all_reduce(sm2[:], sm1[:], channels=128,
                                           reduce_op=bass_isa.ReduceOp.max)
            negC2 = p_l2a.tile([128, 1], dt.float32, tag="negC2", name="negC2")
            nc.vector.tensor_scalar_mul(negC2[:], sm2[:], -1.0)
            nc.sync.dma_start(c2_slab[:].rearrange("(o a) -> o a", o=1),
                              sm2[0:1, 0:1])
            w2all = p_l2a.tile([128, NIT], dt.float32, tag="w2all",
                               name="w2all")
            nc.scalar.activation(w2all[:], s2p[:], AF.Exp, bias=negC2[:])
            # g2 = [w2*h2_64 | 64*w2] single fp8
            for it in range(NIT):
                g2 = p_l2a.tile([128, G2P], dt.float32, tag="g2", name="g2",
                                bufs=2)
                nc.vector.memset(g2[:], 0.0)
                nc.vector.tensor_scalar_mul(g2[:, 0:NCLS], ps2l[it][:],
                                            w2all[:, it:it + 1])
                nc.vector.tensor_scalar_mul(g2[:, NCLS:NCLS + 1],
                                            w2all[:, it:it + 1], SC2)
                g2h = p_l2a.tile([128, G2P], dt.float8e4, tag="g2h",
                                 name="g2h", bufs=2)
                nc.scalar.activation(g2h[:], g2[:], AF.Copy)
                nc.sync.dma_start(g2_slab[it // 2, :, it % 2, :], g2h[:])
            nc.gpsimd.collective_compute(
                "AllGather", ALU.bypass, replica_groups=rg,
                ins=[g2_slab[:]], outs=[g2_full[:]])
            nc.gpsimd.collective_compute(
                "AllGather", ALU.bypass, replica_groups=rg,
                ins=[c2_slab[:]], outs=[c2_full[:]])

            # L2 attention: per-block partials combined with exp(C2_b - max)
            with (
                tc.tile_pool(name="g2t", bufs=NJP) as p_g2t,
                tc.tile_pool(name="fin", bufs=1) as p_f,
                tc.tile_pool(name="ps2", bufs=1, space="PSUM") as ps_2,
            ):
                g2all = p_g2t.tile([128, NJP, 2, G2P], dt.float8e4, tag="g2t",
                                   name="g2t", bufs=1)
                g2v = g2_full.rearrange("jp p t f -> p jp t f")
                for tt in range(2):
                    eng = nc.sync if tt == 0 else nc.gpsimd
                    eng.dma_start(g2all[:, :, tt, :], g2v[:, :, tt, :])
                g2tiles = [g2all[:, jp, :, :] for jp in range(NJP)]
                cload = p_f.tile([1, NCORES], dt.float32, tag="cload",
                                 name="cload")
                nc.sync.dma_start(cload[:],
                                  c2_full[:].rearrange("(o a) -> o a", o=1))
                negmx = p_f.tile([1, 1], dt.float32, tag="negmx", name="negmx")
                nc.vector.tensor_reduce(negmx[:], cload[:],
                                        axis=mybir.AxisListType.X,
                                        op=ALU.max, negate=True)
                srow = p_f.tile([1, NCORES], dt.float32, tag="srow",
                                name="srow")
                nc.scalar.activation(srow[:], cload[:], AF.Exp, bias=negmx[:])
                sbc = p_f.tile([128, NCORES], dt.float32, tag="sbc",
                               name="sbc")
                nc.gpsimd.partition_broadcast(sbc[:], srow[:], channels=128)

                accs = []
                for it in range(NIT):
                    acc = p_f.tile([128, G2P], dt.float32, tag="acc",
                                   name="acc", bufs=NIT)
                    accs.append(acc)
                    for b in range(NCORES):
                        psb = ps_2.tile([128, G2P], dt.float32, tag="ps2",
                                        name="ps2", bufs=2)
                        for k, jp in enumerate((2 * b, 2 * b + 1)):
                            lhs = adjt[jp][:, :, it * 128:(it + 1) * 128]
                            nc.tensor.matmul(psb[:], lhs, g2tiles[jp][:],
                                             start=(k == 0), stop=(k == 1),
                                             perf_mode=DR)
                        if b == 0:
                            nc.vector.tensor_scalar_mul(acc[:], psb[:],
                                                        sbc[:, 0:1])
                        else:
                            nc.vector.scalar_tensor_tensor(
                                acc[:], psb[:], sbc[:, b:b + 1], acc[:],
                                ALU.mult, ALU.add)
                for it in range(NIT):
                    acc = accs[it]
                    r2 = p_f.tile([128, 1], dt.float32, tag="r2", name="r2",
                                  bufs=2)
                    nc.vector.reciprocal(r2[:], acc[:, NCLS:NCLS + 1])
                    z = p_f.tile([128, NCLS], dt.float32, tag="z2", name="z2",
                                 bufs=2)
                    nc.vector.tensor_scalar_mul(z[:], acc[:, 0:NCLS], r2[:])
                    e = p_f.tile([128, NCLS], dt.float32, tag="e2", name="e2",
                                 bufs=2)
                    nc.scalar.activation(e[:], z[:], AF.Exp)
                    nc.vector.tensor_scalar(e[:], e[:], 1.0, -1.0, ALU.min,
                                            ALU.add)
                    o = p_f.tile([128, NCLS], dt.float32, tag="o2", name="o2",
                                 bufs=2)
                    nc.vector.scalar_tensor_tensor(o[:], z[:], 0.0, e[:],
                                                   ALU.max, ALU.add)
                    negm = p_f.tile([128, 1], dt.float32, tag="negm",
                                    name="negm", bufs=2)
                    nc.vector.tensor_reduce(negm[:], o[:],
                                            axis=mybir.AxisListType.X,
                                            op=ALU.max, negate=True)
                    t = p_f.tile([128, NCLS], dt.float32, tag="texp",
                                 name="texp", bufs=2)
                    nc.scalar.activation(t[:], o[:], AF.Exp, bias=negm[:])
                    ssum = p_f.tile([128, 1], dt.float32, tag="ssum",
                                    name="ssum", bufs=2)
                    nc.vector.tensor_reduce(ssum[:], t[:],
                                            axis=mybir.AxisListType.X,
                                            op=ALU.add)
                    lg = p_f.tile([128, 1], dt.float32, tag="lg", name="lg",
                                  bufs=2)
                    nc.scalar.activation(lg[:], ssum[:], AF.Ln)
                    fin = p_f.tile([128, NCLS], dt.float32, tag="fin",
                                   name="fin", bufs=2)
                    nc.vector.tensor_scalar(fin[:], o[:], negm[:], lg[:],
                                            ALU.add, ALU.subtract)
                    nc.sync.dma_start(out_d[it * 128:(it + 1) * 128, :],
                                      fin[:])

    nc.finalize()
    return nc


_CACHE = {}


def _pairb(a):
    hi = a.astype(BF16)
    lo = (a - hi.astype(np.float32)).astype(BF16)
    return hi, lo


def prepare_inputs(x, adj, W_heads, a_heads, W_out, a_out):
    """Shard + lay out the full inputs for the 8 cores."""
    x2 = np.asarray(x, np.float32)[0]          # [N, F]
    adj2 = np.asarray(adj)[0]                  # [N, N] int32
    W3 = np.asarray(W_heads, np.float32).reshape(NH, F, HID)
    a3 = np.asarray(a_heads, np.float32)       # [NH, 2*HID, 1]
    Wo = np.asarray(W_out, np.float32).reshape(GH, NCLS)
    ao = np.asarray(a_out, np.float32)         # [2*NCLS, 1]

    # fold the edge-score projection into the weights: s2 = x @ (W @ a2)
    u = np.einsum("hfo,ho->hf", W3.astype(np.float64),
                  a3[:, HID:, 0].astype(np.float64)).astype(np.float32)
    u_hi, u_lo = _pairb(u)
    U6 = np.zeros((F, 8), BF16)
    U3 = np.zeros((F, 8), BF16)
    for h in range(NH):
        U6[:, 2 * h] = u_hi[h]
        U6[:, 2 * h + 1] = u_lo[h]
        U3[:, h] = u_hi[h]
    W16 = (SC1 * W3).astype(BF16)
    Wo64 = (SC2 * Wo).astype(BF16)
    ao2 = np.ascontiguousarray(ao[NCLS:, 0]).reshape(1, NCLS)
    xT = np.ascontiguousarray(x2.T)            # [F, N]
    adj8 = adj2.astype(E4)                     # exact 0/1

    in_maps = []
    for c in range(NCORES):
        sl = slice(c * SLAB, (c + 1) * SLAB)
        xbh, xbl = _pairb(np.ascontiguousarray(xT[:, sl]))
        in_maps.append({
            "adjT8": np.ascontiguousarray(adj8[sl, :].T),
            "xT_hi": xbh, "xT_lo": xbl,
            "U6": U6, "U3": U3,
            "W16": W16, "Wo64": Wo64,
            "ao2": ao2,
        })
    return in_maps


def kernel(x, adj, W_heads, a_heads, W_out, a_out):
    if "nc" not in _CACHE:
        # touch the devices once so any residual bad state from a previous
        # process surfaces (and clears) before the real run
        try:
            import jax
            jax.block_until_ready(jax.numpy.zeros(8))
        except Exception:
            pass
        _CACHE["nc"] = build()
    nc = _CACHE["nc"]
    in_maps = prepare_inputs(x, adj, W_heads, a_heads, W_out, a_out)
    res = run_bass_kernel_spmd(nc, in_maps, list(range(NCORES)))
    out = np.concatenate([res.results[c]["out"] for c in range(NCORES)], axis=0)
    return out.reshape(1, N, NCLS)
